# revision 30
# baseline (speedup 1.0000x reference)
"""Trainium2 Bass kernel for a dense transformer block (B=8, S=2048, D=768, H=3072).

Sharding: pure data-parallel over batch -- one batch element per NeuronCore.

All GEMMs run as fp8e4m3 DoubleRow matmuls (0.5 PE cycles per output row while
contracting 256 -- 4x the fp32r rate). Weights are quantized host-side with
power-of-2 scales (64x for D-sided weights, 128x for Wproj) so their sigma sits
mid-range in e4m3. Accuracy is recovered where it matters:
  - MLP fc: 3 passes  (W8@h8 + dW8@h8 + W8@r8), dW8/r8 = quantized residuals
  - MLP proj: 2 passes (P8@m8 + dP8@m8)
  - attention (qkv/scores/av/wo): single pass; softmax's diffuse weights make
    it insensitive to fp8 noise. exp is computed shifted (exp(s-2)) to fit
    e4m3's max-240 range.
Residual stream, LN statistics and all PSUM accumulation stay fp32/bf16.

Structure tricks:
  - LN outputs are transposed feature-major via XBAR dma_start_transpose
    (bf16, ~0.7us/tile on the SP/ACT HWDGE queues) instead of PE transposes
    + per-tile PSUM evacuation.
  - rsqrt for both LNs is exp(-0.5*ln(v+eps)) so the whole attention phase
    stays in ACT's natural_log_exp table -- no per-chunk table reloads.
  - qT8 is produced chunk-by-chunk inside the attention loop.

Engine balance (GPSIMD has no PSUM port, so PSUM consumers split ACT/DVE):
  ACT : exp, gelu, ln/exp-rsqrt, qT8 + v8 psum writes, half the transposes
  DVE : LN stats, kT8 psum writes, x2/out fused scale+add (stt), y8, rz
  Pool: LN normalize (SBUF), hT8/h2T8 fp8 copies + r8 residual, Wproj DMA
  SP  : x loads, Wq/Wk/Wo/Wfc DMA, half the transposes, out stores
"""

import numpy as np

P = 128
S, D, H = 2048, 768, 3072
DT = D // P            # 6 d-tiles
HT = H // P            # 24 h-tiles
ST = S // P            # 16 token tiles
CH = 512               # s1 chunk width
NCH = S // CH          # 4 chunks
TPC = CH // P          # 4 token tiles per chunk
D2C = 384              # half-D psum tile
EPS = 1e-5
N_CORES = 8

SW = 64.0              # host scale for Wq/Wk/Wv/Wo/Wfc
SWP = 128.0            # host scale for Wproj
EXP_SHIFT = -2.0       # exp(s + EXP_SHIFT): keeps e8 below e4m3 max (240)
ZINV = 1.0 / 32.0      # "ones" matmul value; rz = 32/Z keeps y8 ~ sigma 1

WEIGHT_NAMES = [
    "ln1_g", "ln1_b", "ln2_g", "ln2_b",
    "Wq", "bq", "Wk", "bk", "Wv", "bv", "Wo", "bo",
    "Wfc", "bfc", "Wproj", "bproj",
]

_CACHE = {}


def host_inputs(inputs):
    """Quantize weights host-side; returns the per-core shared tensor map."""
    import ml_dtypes
    F8 = ml_dtypes.float8_e4m3
    f32 = lambda a: np.ascontiguousarray(np.asarray(a, dtype=np.float32))
    q8 = lambda a: np.ascontiguousarray(np.asarray(a, dtype=np.float32).astype(F8))
    d = {nm: f32(inputs[nm]) for nm in WEIGHT_NAMES}
    m = {}
    for nm in ("Wq", "Wk", "Wv", "Wo"):
        m[nm + "8"] = q8(SW * d[nm])
    wfc8 = q8(SW * d["Wfc"])
    m["Wfc8"] = wfc8
    m["dWfc8"] = q8(SW * d["Wfc"] - wfc8.astype(np.float32))
    wpr8 = q8(SWP * d["Wproj"])
    m["Wpr8"] = wpr8
    m["dWpr8"] = q8(SWP * d["Wproj"] - wpr8.astype(np.float32))
    m["bq2"] = f32(2.0 * d["bq"])
    m["bk2"] = f32(2.0 * d["bk"])
    for nm in ("bv", "bo", "bfc", "bproj", "ln1_g", "ln1_b", "ln2_g", "ln2_b"):
        m[nm] = d[nm]
    return m


def build_flags(inputs):
    zb = all(float(np.abs(np.asarray(inputs[nm])).max()) == 0.0
             for nm in ("bv", "bo", "bproj"))
    lt = (float(np.abs(np.asarray(inputs["ln1_g"]) - 1.0).max()) == 0.0
          and float(np.abs(np.asarray(inputs["ln2_g"]) - 1.0).max()) == 0.0
          and float(np.abs(np.asarray(inputs["ln1_b"])).max()) == 0.0
          and float(np.abs(np.asarray(inputs["ln2_b"])).max()) == 0.0)
    return zb, lt


DRAM_SPECS = (
    [("Wq8", [D, D]), ("Wk8", [D, D]), ("Wv8", [D, D]), ("Wo8", [D, D]),
     ("Wfc8", [D, H]), ("dWfc8", [D, H]), ("Wpr8", [H, D]), ("dWpr8", [H, D])],
    [("bq2", [D]), ("bk2", [D]), ("bv", [D]), ("bo", [D]), ("bfc", [H]),
     ("bproj", [D]), ("ln1_g", [D]), ("ln1_b", [D]), ("ln2_g", [D]),
     ("ln2_b", [D])],
)


def _build(zero_bias=True, ln_trivial=True):
    import concourse.bass as bass
    import concourse.tile as tile
    from concourse import bacc, mybir
    from contextlib import ExitStack

    F = mybir.dt.float32
    BF = mybir.dt.bfloat16
    F8 = mybir.dt.float8e4
    AF = mybir.ActivationFunctionType
    OP = mybir.AluOpType
    DR = mybir.MatmulPerfMode.DoubleRow

    nc = bacc.Bacc(None, target_bir_lowering=False)

    x_d = nc.dram_tensor("x", [S, D], F, kind="ExternalInput")
    w_d = {}
    for nm, shp in DRAM_SPECS[0]:
        w_d[nm] = nc.dram_tensor(nm, shp, F8, kind="ExternalInput")
    for nm, shp in DRAM_SPECS[1]:
        w_d[nm] = nc.dram_tensor(nm, shp, F, kind="ExternalInput")
    out_d = nc.dram_tensor("out", [S, D], F, kind="ExternalOutput")

    def bcast_ap(dram_t, n_part=P):
        ap = dram_t.ap()
        return bass.AP(tensor=ap.tensor, offset=ap.offset, ap=[[0, n_part]] + list(ap.ap))

    inv_sqrt_d = 1.0 / float(np.sqrt(np.float32(D)))

    def wload(dst, name, eng=None, dtile=None):
        """Load weight [K, N] -> [P, K/P, N]; optionally one K-tile slice."""
        src = w_d[name].ap()
        if dtile is not None:
            src = src[dtile * P:(dtile + 1) * P, :]
            dst = dst[:, dtile:dtile + 1, :]
        (eng or nc.gpsimd).dma_start(dst, src.rearrange("(t p) n -> p t n", p=P))

    def rsqrt(rs, var_col, eps_t):
        # rs = exp(-0.5*ln(var+eps)); ln+exp share ACT's natural_log_exp
        # table with exp/identity, so the attention loop never reloads tables.
        nc.scalar.activation(out=rs, in_=var_col, func=AF.Ln, bias=eps_t, scale=1.0)
        nc.scalar.activation(out=rs, in_=rs, func=AF.Exp, scale=-0.5)

    with tile.TileContext(nc) as tc, ExitStack() as ctx:
        singles = ctx.enter_context(tc.tile_pool(name="singles", bufs=1))

        # persistent constants
        ones8 = singles.tile([P, 2, P], F8)
        nc.vector.memset(ones8, ZINV)
        eps_t = singles.tile([P, 1], F)
        nc.vector.memset(eps_t, EPS)
        eshift_t = singles.tile([P, 1], F)
        nc.vector.memset(eshift_t, EXP_SHIFT)
        cols = {}
        for nm, n in [("bq2", DT), ("bk2", DT), ("bfc", HT)]:
            t = singles.tile([P, n], F, tag=f"col_{nm}", name=f"col_{nm}")
            nc.sync.dma_start(t, w_d[nm].ap().rearrange("(t p) -> p t", p=P))
            cols[nm] = t
        ln_bc = {}
        if not ln_trivial:
            for nm in ("ln1_g", "ln1_b", "ln2_g", "ln2_b"):
                t = singles.tile([P, D], F, tag=f"bc_{nm}", name=f"bc_{nm}")
                nc.gpsimd.dma_start(out=t, in_=bcast_ap(w_d[nm]))
                ln_bc[nm] = t

        # persistent activation tensors (allocated lazily -- see pool notes)
        live = ctx.enter_context(tc.tile_pool(name="live", bufs=1))

        # wfc opens before kqv so kqv can close first (LIFO pool order)
        wmlp_ctx = ExitStack()
        wmlp = wmlp_ctx.enter_context(tc.tile_pool(name="wfc", bufs=1))

        attn_ctx = ExitStack()
        kqv = attn_ctx.enter_context(tc.tile_pool(name="kqv", bufs=1))
        kT8 = kqv.tile([P, DT, S], F8)
        qT8 = kqv.tile([P, DT, S], F8)
        v8 = kqv.tile([P, ST, D], F8)

        def ln_chain(pool, src_ap, g_bc, b_bc):
            """bn_stats -> rsqrt -> normalized bf16 tile (token-major)."""
            stats = pool.tile([P, 3, 6], F, tag="st")
            for i in range(3):
                nc.vector.bn_stats(out=stats[:, i, :],
                                   in_=src_ap[:, i * 256:(i + 1) * 256])
            mv = pool.tile([P, 2], F, tag="mv")
            nc.vector.bn_aggr(out=mv, in_=stats)
            rs = pool.tile([P, 1], F, tag="rs")
            rsqrt(rs, mv[:, 1:2], eps_t)
            h_t = pool.tile([P, D], BF, tag="ht")
            nc.gpsimd.tensor_scalar(out=h_t, in0=src_ap, scalar1=mv[:, 0:1],
                                    scalar2=rs, op0=OP.subtract, op1=OP.mult)
            if g_bc is not None:
                nc.gpsimd.tensor_tensor(out=h_t, in0=h_t, in1=g_bc, op=OP.mult)
                nc.gpsimd.tensor_tensor(out=h_t, in0=h_t, in1=b_bc, op=OP.add)
            return h_t

        # ---------------- Phase A: LN1 -> hT8; k,v ----------------
        with (
            tc.tile_pool(name="phA", bufs=3) as phA,
            tc.tile_pool(name="htr", bufs=3) as htrp,
            tc.tile_pool(name="hT", bufs=1) as hTp,
            tc.tile_pool(name="wqkv", bufs=1) as wqkv,
            tc.tile_pool(name="psAb", bufs=3, space="PSUM") as psAb,
        ):
            # hT8 and Wq survive into the attention block (per-chunk qT8)
            hT8 = kqv.tile([P, DT, S], F8)
            bv_bc = None
            if not zero_bias:
                bv_bc = hTp.tile([P, D], F)
                nc.gpsimd.dma_start(out=bv_bc, in_=bcast_ap(w_d["bv"]))
            wv8_t = wqkv.tile([P, DT, D], F8, tag="wv")
            wload(wv8_t, "Wv8")                       # Pool: needed first
            wq8_t = kqv.tile([P, DT, D], F8, tag="wq")
            wk8_t = wqkv.tile([P, DT, D], F8, tag="wk")
            wo8_t = kqv.tile([P, DT, D], F8)

            for st in range(ST + 2):
                if st == 2:
                    # q/k/o weights ride the SP queue behind the first x tiles
                    wload(wk8_t, "Wk8", eng=nc.sync)
                    wload(wq8_t, "Wq8", eng=nc.sync)
                    wload(wo8_t, "Wo8", eng=nc.sync)
                if st >= 2:
                    sv = st - 2
                    for dc in range(2):
                        ps = psAb.tile([P, D2C], F, tag="mmv")
                        for pr in range(DT // 2):
                            nc.tensor.matmul(
                                ps,
                                hT8[:, 2 * pr:2 * pr + 2, sv * P:(sv + 1) * P],
                                wv8_t[:, 2 * pr:2 * pr + 2, dc * D2C:(dc + 1) * D2C],
                                start=(pr == 0), stop=(pr == DT // 2 - 1),
                                perf_mode=DR)
                        dsl = slice(dc * D2C, (dc + 1) * D2C)
                        if zero_bias:
                            nc.scalar.activation(out=v8[:, sv, dsl], in_=ps,
                                                 func=AF.Identity, scale=1.0 / SW)
                        else:
                            nc.vector.scalar_tensor_tensor(
                                out=v8[:, sv, dsl], in0=ps, scalar=1.0 / SW,
                                in1=bv_bc[:, dsl], op0=OP.mult, op1=OP.add)

                if st < ST:
                    x_t = phA.tile([P, D], F, tag="xt")
                    nc.sync.dma_start(x_t, x_d.ap()[st * P:(st + 1) * P, :])
                    h_t = ln_chain(phA, x_t,
                                   ln_bc.get("ln1_g"), ln_bc.get("ln1_b"))
                    htr = htrp.tile([P, DT, P], BF, tag="htr")
                    eng = nc.sync if st % 2 == 0 else nc.scalar
                    eng.dma_start_transpose(htr, h_t)
                    nc.gpsimd.tensor_copy(
                        out=hT8[:, :, st * P:(st + 1) * P], in_=htr)
            # k: phase B needs kT8 chunks in order; DVE evacuates
            for sc in range(NCH):
                for dtp in range(DT):
                    ps = psAb.tile([P, CH], F, tag="mm")
                    for pr in range(DT // 2):
                        nc.tensor.matmul(
                            ps,
                            wk8_t[:, 2 * pr:2 * pr + 2, dtp * P:(dtp + 1) * P],
                            hT8[:, 2 * pr:2 * pr + 2, sc * CH:(sc + 1) * CH],
                            start=(pr == 0), stop=(pr == DT // 2 - 1),
                            perf_mode=DR)
                    nc.vector.tensor_scalar(out=kT8[:, dtp, sc * CH:(sc + 1) * CH],
                                            in0=ps, scalar1=2.0 / SW,
                                            scalar2=cols["bk2"][:, dtp:dtp + 1],
                                            op0=OP.mult, op1=OP.add)

        # fc weights + persistent attention-output tensors
        wfc8_t = wmlp.tile([P, DT, H], F8)
        dwfc8_t = wmlp.tile([P, DT, H], F8)
        x2bf = live.tile([P, ST, D], BF)         # post-attn residual (bf16)
        h2T8 = live.tile([P, DT, S], F8)         # LN2 out, feature-major fp8
        r8 = live.tile([P, DT, S], F8)           # fp8 residual of h2T
        bo_bc = None
        if not zero_bias:
            bo_bc = live.tile([P, D], F)
            nc.gpsimd.dma_start(out=bo_bc, in_=bcast_ap(w_d["bo"]))

        # ---------------- Phase B/C: attention + fused LN2 ----------------
        NPR = ST // 2  # 8 s2 pairs
        with (
            tc.tile_pool(name="phC", bufs=2) as phC,
            tc.tile_pool(name="h2tr", bufs=3) as h2trp,
            tc.tile_pool(name="e8p", bufs=1) as e8p,
            tc.tile_pool(name="yt", bufs=2) as ytp,
            tc.tile_pool(name="ps_a", bufs=2, space="PSUM") as ps_a,
            tc.tile_pool(name="ps_y", bufs=6, space="PSUM") as ps_y,
        ):
            y8_sbs = [None] * NCH
            for sc in range(NCH + 1):
                if sc < NCH:
                    # qT8 for this chunk (ACT evacuates; identity rides the
                    # exp table so no act-table reload)
                    for dtp in range(DT):
                        ps = ps_a.tile([P, CH], F, tag="sc", name="ps_q")
                        for pr in range(DT // 2):
                            nc.tensor.matmul(
                                ps,
                                wq8_t[:, 2 * pr:2 * pr + 2, dtp * P:(dtp + 1) * P],
                                hT8[:, 2 * pr:2 * pr + 2, sc * CH:(sc + 1) * CH],
                                start=(pr == 0), stop=(pr == DT // 2 - 1),
                                perf_mode=DR)
                        nc.scalar.activation(out=qT8[:, dtp, sc * CH:(sc + 1) * CH],
                                             in_=ps, func=AF.Identity,
                                             bias=cols["bq2"][:, dtp:dtp + 1],
                                             scale=2.0 / SW)
                    e8c = e8p.tile([P, ST, CH], F8, tag="e8")
                    ps_ys = [ps_y.tile([P, CH], F, tag="y", name=f"ps_y{i}")
                             for i in range(DT)]
                    for pi in range(NPR + 1):
                        if pi < NPR:
                            for j in range(2):
                                st2 = 2 * pi + j
                                ps = ps_a.tile([P, CH], F, tag="sc")
                                for pr in range(DT // 2):
                                    nc.tensor.matmul(
                                        ps,
                                        kT8[:, 2 * pr:2 * pr + 2, st2 * P:(st2 + 1) * P],
                                        qT8[:, 2 * pr:2 * pr + 2, sc * CH:(sc + 1) * CH],
                                        start=(pr == 0), stop=(pr == DT // 2 - 1),
                                        perf_mode=DR)
                                nc.scalar.activation(out=e8c[:, st2, :], in_=ps,
                                                     func=AF.Exp, bias=eshift_t,
                                                     scale=inv_sqrt_d / 4.0)
                        if pi >= 1:
                            pp = pi - 1
                            for dtp in range(DT):
                                nc.tensor.matmul(
                                    ps_ys[dtp],
                                    v8[:, 2 * pp:2 * pp + 2, dtp * P:(dtp + 1) * P],
                                    e8c[:, 2 * pp:2 * pp + 2, :],
                                    start=(pp == 0), stop=(pp == NPR - 1),
                                    perf_mode=DR)
                    ps_zt = ps_a.tile([P, CH], F, tag="sc", name="ps_zt")
                    for pp in range(NPR):
                        nc.tensor.matmul(ps_zt, ones8, e8c[:, 2 * pp:2 * pp + 2, :],
                                         start=(pp == 0), stop=(pp == NPR - 1),
                                         perf_mode=DR)
                    rz = phC.tile([P, CH], F, tag="rz")
                    nc.vector.reciprocal(out=rz, in_=ps_zt)
                    y8_sb = ytp.tile([P, DT, CH], F8, tag="yt")
                    for dtp in range(DT):
                        nc.vector.tensor_tensor(out=y8_sb[:, dtp], in0=ps_ys[dtp],
                                                in1=rz, op=OP.mult)
                    y8_sbs[sc] = y8_sb

                if sc >= 1:
                    cc = sc - 1
                    y8_sb = y8_sbs[cc]
                    for su in range(TPC):
                        st = cc * TPC + su
                        x_t = phC.tile([P, D], F, tag="xt3")
                        nc.sync.dma_start(x_t, x_d.ap()[st * P:(st + 1) * P, :])
                        # one Wfc d-tile slice per su rides the SP queue
                        if cc < 3:
                            k = cc * TPC + su
                            if k < DT:
                                wload(wfc8_t, "Wfc8", eng=nc.sync, dtile=k)
                            elif k < 2 * DT:
                                wload(dwfc8_t, "dWfc8", eng=nc.sync, dtile=k - DT)
                        for dc in range(2):
                            ps = ps_y.tile([P, D2C], F, tag="y", name="ps_o")
                            for pr in range(DT // 2):
                                nc.tensor.matmul(
                                    ps,
                                    y8_sb[:, 2 * pr:2 * pr + 2, su * P:(su + 1) * P],
                                    wo8_t[:, 2 * pr:2 * pr + 2, dc * D2C:(dc + 1) * D2C],
                                    start=(pr == 0), stop=(pr == DT // 2 - 1),
                                    perf_mode=DR)
                            dsl = slice(dc * D2C, (dc + 1) * D2C)
                            nc.vector.scalar_tensor_tensor(
                                out=x2bf[:, st, dsl], in0=ps,
                                scalar=1.0 / (32.0 * SW), in1=x_t[:, dsl],
                                op0=OP.mult, op1=OP.add)
                        if not zero_bias:
                            nc.vector.tensor_tensor(out=x2bf[:, st, :],
                                                    in0=x2bf[:, st, :],
                                                    in1=bo_bc, op=OP.add)
                        h2_t = ln_chain(phC, x2bf[:, st, :],
                                        ln_bc.get("ln2_g"), ln_bc.get("ln2_b"))
                        # XBAR transpose needs a dense destination (strided
                        # dest is silently wrong on HW) -- land in h2tr, then
                        # Pool writes the fp8 copy + residual.
                        h2tr = h2trp.tile([P, DT, P], BF, tag="h2tr")
                        eng = nc.sync if su % 2 == 0 else nc.scalar
                        eng.dma_start_transpose(h2tr, h2_t)
                        tsl = slice(st * P, (st + 1) * P)
                        nc.gpsimd.tensor_copy(out=h2T8[:, :, tsl], in_=h2tr)
                        nc.gpsimd.tensor_tensor(out=r8[:, :, tsl], in0=h2tr,
                                                in1=h2T8[:, :, tsl],
                                                op=OP.subtract)

        attn_ctx.close()

        # ---------------- Phase D: MLP ----------------
        wpr_pool = wmlp_ctx.enter_context(tc.tile_pool(name="wpr", bufs=1))
        wpr8_t = wpr_pool.tile([P, HT, D], F8)
        wload(wpr8_t, "Wpr8")
        dwpr8_t = wpr_pool.tile([P, HT, D], F8)
        wload(dwpr8_t, "dWpr8")
        bp_bc = None
        if not zero_bias:
            bp_bc = wpr_pool.tile([P, D], F)
            nc.gpsimd.dma_start(out=bp_bc, in_=bcast_ap(w_d["bproj"]))
        with (
            tc.tile_pool(name="phD", bufs=2) as phD,
            tc.tile_pool(name="mt", bufs=2) as mtp,
            tc.tile_pool(name="ps_u", bufs=4, space="PSUM") as ps_u,
            tc.tile_pool(name="ps_p", bufs=4, space="PSUM") as ps_p,
        ):
            m8_sbs = [None] * NCH

            def fc(sc):
                csl = slice(sc * CH, (sc + 1) * CH)
                m8_sb = mtp.tile([P, HT, CH], F8, tag="mt")
                for ht in range(HT):
                    ps = ps_u.tile([P, CH], F, tag="u")
                    hsl = slice(ht * P, (ht + 1) * P)
                    passes = [(wfc8_t, h2T8), (dwfc8_t, h2T8), (wfc8_t, r8)]
                    for pa, (wt, act) in enumerate(passes):
                        for pr in range(DT // 2):
                            nc.tensor.matmul(
                                ps,
                                wt[:, 2 * pr:2 * pr + 2, hsl],
                                act[:, 2 * pr:2 * pr + 2, csl],
                                start=(pa == 0 and pr == 0),
                                stop=(pa == len(passes) - 1 and pr == DT // 2 - 1),
                                perf_mode=DR)
                    nc.scalar.activation(out=m8_sb[:, ht], in_=ps, func=AF.Gelu,
                                         bias=cols["bfc"][:, ht:ht + 1], scale=1.0 / SW)
                m8_sbs[sc] = m8_sb

            def proj(sc):
                m8_sb = m8_sbs[sc]
                for su in range(TPC):
                    st = sc * TPC + su
                    o2_t = phD.tile([P, D], F, tag="o2")
                    for dc in range(2):
                        ps = ps_p.tile([P, D2C], F, tag="o2p")
                        for pa, wt in enumerate((wpr8_t, dwpr8_t)):
                            for tr_ in range(HT // 2):
                                nc.tensor.matmul(
                                    ps,
                                    m8_sb[:, 2 * tr_:2 * tr_ + 2, su * P:(su + 1) * P],
                                    wt[:, 2 * tr_:2 * tr_ + 2, dc * D2C:(dc + 1) * D2C],
                                    start=(pa == 0 and tr_ == 0),
                                    stop=(pa == 1 and tr_ == HT // 2 - 1),
                                    perf_mode=DR)
                        dsl = slice(dc * D2C, (dc + 1) * D2C)
                        nc.vector.scalar_tensor_tensor(
                            out=o2_t[:, dsl], in0=ps, scalar=1.0 / SWP,
                            in1=x2bf[:, st, dsl], op0=OP.mult, op1=OP.add)
                    if not zero_bias:
                        nc.vector.tensor_tensor(out=o2_t, in0=o2_t, in1=bp_bc,
                                                op=OP.add)
                    nc.sync.dma_start(out_d.ap()[st * P:(st + 1) * P, :], o2_t)

            fc(0)
            for sc in range(1, NCH):
                fc(sc)
                proj(sc - 1)
            proj(NCH - 1)
        wmlp_ctx.close()

    return nc


def _get_nc(zero_bias=True, ln_trivial=True):
    key = ("nc", zero_bias, ln_trivial)
    if key not in _CACHE:
        nc = _build(zero_bias, ln_trivial)
        nc.compile()
        _CACHE[key] = nc
    return _CACHE[key]


TRACE = False


def kernel(**inputs):
    from concourse.bass_utils import run_bass_kernel_spmd

    zb, lt = build_flags(inputs)
    nc = _get_nc(zb, lt)
    x = np.asarray(inputs["x"], dtype=np.float32)
    base = host_inputs(inputs)
    in_maps = [dict(base, x=np.ascontiguousarray(x[b])) for b in range(N_CORES)]
    res = run_bass_kernel_spmd(nc, in_maps, core_ids=list(range(N_CORES)), trace=TRACE)
    _CACHE["last_res"] = res
    return np.stack([res.results[b]["out"] for b in range(N_CORES)], axis=0)


# revision 34
# speedup vs baseline: 1.0700x; 1.0700x over previous
"""Trainium2 Bass kernel for a dense transformer block (B=8, S=2048, D=768, H=3072).

Sharding: pure data-parallel over batch -- one batch element per NeuronCore.

All GEMMs run as fp8e4m3 DoubleRow matmuls (0.5 PE cycles per output row while
contracting 256 -- 4x the fp32r rate). Weights are quantized host-side with
power-of-2 scales (64x for D-sided weights, 128x for Wproj) so their sigma sits
mid-range in e4m3. Accuracy is recovered where it matters:
  - MLP fc: 3 passes  (W8@h8 + dW8@h8 + W8@r8), dW8/r8 = quantized residuals
  - MLP proj: 2 passes (P8@m8 + dP8@m8)
  - attention (qkv/scores/av/wo): single pass; softmax's diffuse weights make
    it insensitive to fp8 noise. exp is computed shifted (exp(s-2)) to fit
    e4m3's max-240 range.
Residual stream, LN statistics and all PSUM accumulation stay fp32/bf16.

Structure tricks:
  - LN outputs are transposed feature-major via XBAR dma_start_transpose
    (bf16, ~0.7us/tile on the SP/ACT HWDGE queues) instead of PE transposes
    + per-tile PSUM evacuation.
  - rsqrt for both LNs is exp(-0.5*ln(v+eps)) so the whole attention phase
    stays in ACT's natural_log_exp table -- no per-chunk table reloads.
  - qT8 is produced chunk-by-chunk inside the attention loop.

Engine balance (GPSIMD has no PSUM port, so PSUM consumers split ACT/DVE):
  ACT : exp, gelu, ln/exp-rsqrt, qT8 + v8 psum writes, half the transposes
  DVE : LN stats, kT8 psum writes, x2/out fused scale+add (stt), y8, rz
  Pool: LN normalize (SBUF), hT8/h2T8 fp8 copies + r8 residual, Wproj DMA
  SP  : x loads, Wq/Wk/Wo/Wfc DMA, half the transposes, out stores
"""

import numpy as np

P = 128
S, D, H = 2048, 768, 3072
DT = D // P            # 6 d-tiles
HT = H // P            # 24 h-tiles
ST = S // P            # 16 token tiles
CH = 512               # s1 chunk width
NCH = S // CH          # 4 chunks
TPC = CH // P          # 4 token tiles per chunk
D2C = 384              # half-D psum tile
EPS = 1e-5
N_CORES = 8

SW = 64.0              # host scale for Wq/Wk/Wv/Wo/Wfc
SWP = 128.0            # host scale for Wproj
EXP_SHIFT = -2.0       # exp(s + EXP_SHIFT): keeps e8 below e4m3 max (240)
ZINV = 1.0 / 32.0      # "ones" matmul value; rz = 32/Z keeps y8 ~ sigma 1

WEIGHT_NAMES = [
    "ln1_g", "ln1_b", "ln2_g", "ln2_b",
    "Wq", "bq", "Wk", "bk", "Wv", "bv", "Wo", "bo",
    "Wfc", "bfc", "Wproj", "bproj",
]

_CACHE = {}


def host_inputs(inputs):
    """Quantize weights host-side; returns the per-core shared tensor map."""
    import ml_dtypes
    F8 = ml_dtypes.float8_e4m3
    f32 = lambda a: np.ascontiguousarray(np.asarray(a, dtype=np.float32))
    q8 = lambda a: np.ascontiguousarray(np.asarray(a, dtype=np.float32).astype(F8))
    d = {nm: f32(inputs[nm]) for nm in WEIGHT_NAMES}
    m = {}
    for nm in ("Wq", "Wk", "Wv", "Wo"):
        m[nm + "8"] = q8(SW * d[nm])
    wfc8 = q8(SW * d["Wfc"])
    m["Wfc8"] = wfc8
    m["dWfc8"] = q8(SW * d["Wfc"] - wfc8.astype(np.float32))
    wpr8 = q8(SWP * d["Wproj"])
    m["Wpr8"] = wpr8
    m["dWpr8"] = q8(SWP * d["Wproj"] - wpr8.astype(np.float32))
    m["bq2"] = f32(2.0 * d["bq"])
    m["bk2"] = f32(2.0 * d["bk"])
    for nm in ("bv", "bo", "bfc", "bproj", "ln1_g", "ln1_b", "ln2_g", "ln2_b"):
        m[nm] = d[nm]
    return m


def build_flags(inputs):
    zb = all(float(np.abs(np.asarray(inputs[nm])).max()) == 0.0
             for nm in ("bv", "bo", "bproj"))
    lt = (float(np.abs(np.asarray(inputs["ln1_g"]) - 1.0).max()) == 0.0
          and float(np.abs(np.asarray(inputs["ln2_g"]) - 1.0).max()) == 0.0
          and float(np.abs(np.asarray(inputs["ln1_b"])).max()) == 0.0
          and float(np.abs(np.asarray(inputs["ln2_b"])).max()) == 0.0)
    return zb, lt


DRAM_SPECS = (
    [("Wq8", [D, D]), ("Wk8", [D, D]), ("Wv8", [D, D]), ("Wo8", [D, D]),
     ("Wfc8", [D, H]), ("dWfc8", [D, H]), ("Wpr8", [H, D]), ("dWpr8", [H, D])],
    [("bq2", [D]), ("bk2", [D]), ("bv", [D]), ("bo", [D]), ("bfc", [H]),
     ("bproj", [D]), ("ln1_g", [D]), ("ln1_b", [D]), ("ln2_g", [D]),
     ("ln2_b", [D])],
)


def _build(zero_bias=True, ln_trivial=True):
    import concourse.bass as bass
    import concourse.tile as tile
    from concourse import bacc, mybir
    from contextlib import ExitStack

    F = mybir.dt.float32
    BF = mybir.dt.bfloat16
    F8 = mybir.dt.float8e4
    AF = mybir.ActivationFunctionType
    OP = mybir.AluOpType
    DR = mybir.MatmulPerfMode.DoubleRow

    nc = bacc.Bacc(None, target_bir_lowering=False)

    x_d = nc.dram_tensor("x", [S, D], F, kind="ExternalInput")
    w_d = {}
    for nm, shp in DRAM_SPECS[0]:
        w_d[nm] = nc.dram_tensor(nm, shp, F8, kind="ExternalInput")
    for nm, shp in DRAM_SPECS[1]:
        w_d[nm] = nc.dram_tensor(nm, shp, F, kind="ExternalInput")
    out_d = nc.dram_tensor("out", [S, D], F, kind="ExternalOutput")

    def bcast_ap(dram_t, n_part=P):
        ap = dram_t.ap()
        return bass.AP(tensor=ap.tensor, offset=ap.offset, ap=[[0, n_part]] + list(ap.ap))

    inv_sqrt_d = 1.0 / float(np.sqrt(np.float32(D)))

    def wload(dst, name, eng=None, dtile=None):
        """Load weight [K, N] -> [P, K/P, N]; optionally one K-tile slice."""
        src = w_d[name].ap()
        if dtile is not None:
            src = src[dtile * P:(dtile + 1) * P, :]
            dst = dst[:, dtile:dtile + 1, :]
        (eng or nc.gpsimd).dma_start(dst, src.rearrange("(t p) n -> p t n", p=P))

    I32 = mybir.dt.int32
    RSQRT_MAGIC = 0x5F3759DF

    def rsqrt(pool, var_col):
        # Quake rsqrt on the Pool ALU (magic bit-hack + one Newton step,
        # |err| < 0.2% -- far below the fp8 noise floor). Keeps the sqrt off
        # ACT so the whole attention phase runs in one activation table.
        ve = pool.tile([P, 1], F, tag="ve")
        nc.vector.tensor_scalar(out=ve, in0=var_col, scalar1=EPS, scalar2=None,
                                op0=OP.add)
        y0 = pool.tile([P, 1], F, tag="y0")
        nc.vector.tensor_scalar(out=y0[:].bitcast(I32), in0=ve[:].bitcast(I32),
                                scalar1=1, scalar2=None,
                                op0=OP.logical_shift_right)
        nc.vector.tensor_scalar(out=y0[:].bitcast(I32), in0=y0[:].bitcast(I32),
                                scalar1=-1, scalar2=RSQRT_MAGIC,
                                op0=OP.mult, op1=OP.add)
        s = pool.tile([P, 1], F, tag="nsq")
        nc.vector.tensor_tensor(out=s, in0=y0, in1=y0, op=OP.mult)
        nc.vector.tensor_tensor(out=s, in0=s, in1=ve, op=OP.mult)
        nc.vector.tensor_scalar(out=s, in0=s, scalar1=-0.5, scalar2=1.5,
                                op0=OP.mult, op1=OP.add)
        rs = pool.tile([P, 1], F, tag="rs")
        nc.vector.tensor_tensor(out=rs, in0=y0, in1=s, op=OP.mult)
        return rs

    with tile.TileContext(nc) as tc, ExitStack() as ctx:
        singles = ctx.enter_context(tc.tile_pool(name="singles", bufs=1))

        # persistent constants
        ones8 = singles.tile([P, 2, P], F8)
        nc.vector.memset(ones8, ZINV)
        eps_t = singles.tile([P, 1], F)
        nc.vector.memset(eps_t, EPS)
        eshift_t = singles.tile([P, 1], F)
        nc.vector.memset(eshift_t, EXP_SHIFT)
        cols = {}
        for nm, n in [("bq2", DT), ("bk2", DT), ("bfc", HT)]:
            t = singles.tile([P, n], F, tag=f"col_{nm}", name=f"col_{nm}")
            nc.sync.dma_start(t, w_d[nm].ap().rearrange("(t p) -> p t", p=P))
            cols[nm] = t
        ln_bc = {}
        if not ln_trivial:
            for nm in ("ln1_g", "ln1_b", "ln2_g", "ln2_b"):
                t = singles.tile([P, D], F, tag=f"bc_{nm}", name=f"bc_{nm}")
                nc.gpsimd.dma_start(out=t, in_=bcast_ap(w_d[nm]))
                ln_bc[nm] = t

        # persistent activation tensors (allocated lazily -- see pool notes)
        live = ctx.enter_context(tc.tile_pool(name="live", bufs=1))

        # wfc opens before kqv so kqv can close first (LIFO pool order)
        wmlp_ctx = ExitStack()
        wmlp = wmlp_ctx.enter_context(tc.tile_pool(name="wfc", bufs=1))

        attn_ctx = ExitStack()
        kqv = attn_ctx.enter_context(tc.tile_pool(name="kqv", bufs=1))
        kT8 = kqv.tile([P, DT, S], F8)
        qT8 = kqv.tile([P, DT, S], F8)
        v8 = kqv.tile([P, ST, D], F8)

        def ln_chain(pool, src_ap, g_bc, b_bc):
            """bn_stats -> rsqrt -> normalized bf16 tile (token-major)."""
            stats = pool.tile([P, 3, 6], F, tag="st")
            for i in range(3):
                nc.vector.bn_stats(out=stats[:, i, :],
                                   in_=src_ap[:, i * 256:(i + 1) * 256])
            mv = pool.tile([P, 2], F, tag="mv")
            nc.vector.bn_aggr(out=mv, in_=stats)
            rs = rsqrt(pool, mv[:, 1:2])
            h_t = pool.tile([P, D], BF, tag="ht")
            nc.gpsimd.tensor_scalar(out=h_t, in0=src_ap, scalar1=mv[:, 0:1],
                                    scalar2=rs, op0=OP.subtract, op1=OP.mult)
            if g_bc is not None:
                nc.gpsimd.tensor_tensor(out=h_t, in0=h_t, in1=g_bc, op=OP.mult)
                nc.gpsimd.tensor_tensor(out=h_t, in0=h_t, in1=b_bc, op=OP.add)
            return h_t

        # ---------------- Phase A: LN1 -> hT8; k,v ----------------
        with (
            tc.tile_pool(name="phA", bufs=3) as phA,
            tc.tile_pool(name="htr", bufs=3) as htrp,
            tc.tile_pool(name="hT", bufs=1) as hTp,
            tc.tile_pool(name="wqkv", bufs=1) as wqkv,
            tc.tile_pool(name="psAb", bufs=3, space="PSUM") as psAb,
        ):
            # hT8 and Wq survive into the attention block (per-chunk qT8)
            hT8 = kqv.tile([P, DT, S], F8)
            bv_bc = None
            if not zero_bias:
                bv_bc = hTp.tile([P, D], F)
                nc.gpsimd.dma_start(out=bv_bc, in_=bcast_ap(w_d["bv"]))
            wv8_t = wqkv.tile([P, DT, D], F8, tag="wv")
            wload(wv8_t, "Wv8")                       # Pool: needed first
            wq8_t = kqv.tile([P, DT, D], F8, tag="wq")
            wk8_t = wqkv.tile([P, DT, D], F8, tag="wk")
            wo8_t = kqv.tile([P, DT, D], F8)

            for st in range(ST + 2):
                if st == 2:
                    # q/k/o weights ride the SP queue behind the first x tiles
                    wload(wk8_t, "Wk8", eng=nc.sync)
                    wload(wq8_t, "Wq8", eng=nc.sync)
                    wload(wo8_t, "Wo8", eng=nc.sync)
                if st >= 2:
                    sv = st - 2
                    for dc in range(2):
                        ps = psAb.tile([P, D2C], F, tag="mmv")
                        for pr in range(DT // 2):
                            nc.tensor.matmul(
                                ps,
                                hT8[:, 2 * pr:2 * pr + 2, sv * P:(sv + 1) * P],
                                wv8_t[:, 2 * pr:2 * pr + 2, dc * D2C:(dc + 1) * D2C],
                                start=(pr == 0), stop=(pr == DT // 2 - 1),
                                perf_mode=DR)
                        dsl = slice(dc * D2C, (dc + 1) * D2C)
                        if zero_bias:
                            nc.scalar.activation(out=v8[:, sv, dsl], in_=ps,
                                                 func=AF.Identity, scale=1.0 / SW)
                        else:
                            nc.vector.scalar_tensor_tensor(
                                out=v8[:, sv, dsl], in0=ps, scalar=1.0 / SW,
                                in1=bv_bc[:, dsl], op0=OP.mult, op1=OP.add)

                if st < ST:
                    x_t = phA.tile([P, D], F, tag="xt")
                    nc.sync.dma_start(x_t, x_d.ap()[st * P:(st + 1) * P, :])
                    h_t = ln_chain(phA, x_t,
                                   ln_bc.get("ln1_g"), ln_bc.get("ln1_b"))
                    htr = htrp.tile([P, DT, P], BF, tag="htr")
                    eng = nc.sync if st % 2 == 0 else nc.scalar
                    eng.dma_start_transpose(htr, h_t)
                    nc.gpsimd.tensor_copy(
                        out=hT8[:, :, st * P:(st + 1) * P], in_=htr)
            # k: phase B needs kT8 chunks in order; DVE evacuates
            for sc in range(NCH):
                for dtp in range(DT):
                    ps = psAb.tile([P, CH], F, tag="mm")
                    for pr in range(DT // 2):
                        nc.tensor.matmul(
                            ps,
                            wk8_t[:, 2 * pr:2 * pr + 2, dtp * P:(dtp + 1) * P],
                            hT8[:, 2 * pr:2 * pr + 2, sc * CH:(sc + 1) * CH],
                            start=(pr == 0), stop=(pr == DT // 2 - 1),
                            perf_mode=DR)
                    nc.vector.tensor_scalar(out=kT8[:, dtp, sc * CH:(sc + 1) * CH],
                                            in0=ps, scalar1=2.0 / SW,
                                            scalar2=cols["bk2"][:, dtp:dtp + 1],
                                            op0=OP.mult, op1=OP.add)

        # fc weights + persistent attention-output tensors
        wfc8_t = wmlp.tile([P, DT, H], F8)
        dwfc8_t = wmlp.tile([P, DT, H], F8)
        x2bf = live.tile([P, ST, D], BF)         # post-attn residual (bf16)
        h2T8 = live.tile([P, DT, S], F8)         # LN2 out, feature-major fp8
        r8 = live.tile([P, DT, S], F8)           # fp8 residual of h2T
        bo_bc = None
        if not zero_bias:
            bo_bc = live.tile([P, D], F)
            nc.gpsimd.dma_start(out=bo_bc, in_=bcast_ap(w_d["bo"]))

        # ---------------- Phase B/C: attention + fused LN2 ----------------
        NPR = ST // 2  # 8 s2 pairs
        with (
            tc.tile_pool(name="phC", bufs=2) as phC,
            tc.tile_pool(name="h2tr", bufs=3) as h2trp,
            tc.tile_pool(name="e8p", bufs=1) as e8p,
            tc.tile_pool(name="yt", bufs=2) as ytp,
            tc.tile_pool(name="ps_a", bufs=2, space="PSUM") as ps_a,
            tc.tile_pool(name="ps_y", bufs=6, space="PSUM") as ps_y,
        ):
            y8_sbs = [None] * NCH
            for sc in range(NCH + 1):
                if sc < NCH:
                    # qT8 for this chunk (ACT evacuates; identity rides the
                    # exp table so no act-table reload)
                    for dtp in range(DT):
                        ps = ps_a.tile([P, CH], F, tag="sc", name="ps_q")
                        for pr in range(DT // 2):
                            nc.tensor.matmul(
                                ps,
                                wq8_t[:, 2 * pr:2 * pr + 2, dtp * P:(dtp + 1) * P],
                                hT8[:, 2 * pr:2 * pr + 2, sc * CH:(sc + 1) * CH],
                                start=(pr == 0), stop=(pr == DT // 2 - 1),
                                perf_mode=DR)
                        nc.scalar.activation(out=qT8[:, dtp, sc * CH:(sc + 1) * CH],
                                             in_=ps, func=AF.Identity,
                                             bias=cols["bq2"][:, dtp:dtp + 1],
                                             scale=2.0 / SW)
                    e8c = e8p.tile([P, ST, CH], F8, tag="e8")
                    ps_ys = [ps_y.tile([P, CH], F, tag="y", name=f"ps_y{i}")
                             for i in range(DT)]
                    for pi in range(NPR + 1):
                        if pi < NPR:
                            for j in range(2):
                                st2 = 2 * pi + j
                                ps = ps_a.tile([P, CH], F, tag="sc")
                                for pr in range(DT // 2):
                                    nc.tensor.matmul(
                                        ps,
                                        kT8[:, 2 * pr:2 * pr + 2, st2 * P:(st2 + 1) * P],
                                        qT8[:, 2 * pr:2 * pr + 2, sc * CH:(sc + 1) * CH],
                                        start=(pr == 0), stop=(pr == DT // 2 - 1),
                                        perf_mode=DR)
                                nc.scalar.activation(out=e8c[:, st2, :], in_=ps,
                                                     func=AF.Exp, bias=eshift_t,
                                                     scale=inv_sqrt_d / 4.0)
                        if pi >= 1:
                            pp = pi - 1
                            for dtp in range(DT):
                                nc.tensor.matmul(
                                    ps_ys[dtp],
                                    v8[:, 2 * pp:2 * pp + 2, dtp * P:(dtp + 1) * P],
                                    e8c[:, 2 * pp:2 * pp + 2, :],
                                    start=(pp == 0), stop=(pp == NPR - 1),
                                    perf_mode=DR)
                    ps_zt = ps_a.tile([P, CH], F, tag="sc", name="ps_zt")
                    for pp in range(NPR):
                        nc.tensor.matmul(ps_zt, ones8, e8c[:, 2 * pp:2 * pp + 2, :],
                                         start=(pp == 0), stop=(pp == NPR - 1),
                                         perf_mode=DR)
                    rz = phC.tile([P, CH], F, tag="rz")
                    nc.vector.reciprocal(out=rz, in_=ps_zt)
                    y8_sb = ytp.tile([P, DT, CH], F8, tag="yt")
                    for dtp in range(DT):
                        nc.vector.tensor_tensor(out=y8_sb[:, dtp], in0=ps_ys[dtp],
                                                in1=rz, op=OP.mult)
                    y8_sbs[sc] = y8_sb

                if sc >= 1:
                    cc = sc - 1
                    y8_sb = y8_sbs[cc]
                    for su in range(TPC):
                        st = cc * TPC + su
                        x_t = phC.tile([P, D], F, tag="xt3")
                        nc.sync.dma_start(x_t, x_d.ap()[st * P:(st + 1) * P, :])
                        # one Wfc d-tile slice per su rides the SP queue
                        if cc < 3:
                            k = cc * TPC + su
                            if k < DT:
                                wload(wfc8_t, "Wfc8", eng=nc.sync, dtile=k)
                            elif k < 2 * DT:
                                wload(dwfc8_t, "dWfc8", eng=nc.sync, dtile=k - DT)
                        for dc in range(2):
                            ps = ps_y.tile([P, D2C], F, tag="y", name="ps_o")
                            for pr in range(DT // 2):
                                nc.tensor.matmul(
                                    ps,
                                    y8_sb[:, 2 * pr:2 * pr + 2, su * P:(su + 1) * P],
                                    wo8_t[:, 2 * pr:2 * pr + 2, dc * D2C:(dc + 1) * D2C],
                                    start=(pr == 0), stop=(pr == DT // 2 - 1),
                                    perf_mode=DR)
                            dsl = slice(dc * D2C, (dc + 1) * D2C)
                            nc.vector.scalar_tensor_tensor(
                                out=x2bf[:, st, dsl], in0=ps,
                                scalar=1.0 / (32.0 * SW), in1=x_t[:, dsl],
                                op0=OP.mult, op1=OP.add)
                        if not zero_bias:
                            nc.vector.tensor_tensor(out=x2bf[:, st, :],
                                                    in0=x2bf[:, st, :],
                                                    in1=bo_bc, op=OP.add)
                        h2_t = ln_chain(phC, x2bf[:, st, :],
                                        ln_bc.get("ln2_g"), ln_bc.get("ln2_b"))
                        # XBAR transpose needs a dense destination (strided
                        # dest is silently wrong on HW) -- land in h2tr, then
                        # Pool writes the fp8 copy + residual.
                        h2tr = h2trp.tile([P, DT, P], BF, tag="h2tr")
                        eng = nc.sync if su % 2 == 0 else nc.scalar
                        eng.dma_start_transpose(h2tr, h2_t)
                        tsl = slice(st * P, (st + 1) * P)
                        nc.gpsimd.tensor_copy(out=h2T8[:, :, tsl], in_=h2tr)
                        nc.gpsimd.tensor_tensor(out=r8[:, :, tsl], in0=h2tr,
                                                in1=h2T8[:, :, tsl],
                                                op=OP.subtract)

        attn_ctx.close()

        # ---------------- Phase D: MLP ----------------
        wpr_pool = wmlp_ctx.enter_context(tc.tile_pool(name="wpr", bufs=1))
        wpr8_t = wpr_pool.tile([P, HT, D], F8)
        wload(wpr8_t, "Wpr8")
        dwpr8_t = wpr_pool.tile([P, HT, D], F8)
        wload(dwpr8_t, "dWpr8")
        bp_bc = None
        if not zero_bias:
            bp_bc = wpr_pool.tile([P, D], F)
            nc.gpsimd.dma_start(out=bp_bc, in_=bcast_ap(w_d["bproj"]))
        with (
            tc.tile_pool(name="phD", bufs=2) as phD,
            tc.tile_pool(name="mt", bufs=2) as mtp,
            tc.tile_pool(name="ps_u", bufs=4, space="PSUM") as ps_u,
            tc.tile_pool(name="ps_p", bufs=4, space="PSUM") as ps_p,
        ):
            m8_sbs = [None] * NCH

            def fc(sc):
                csl = slice(sc * CH, (sc + 1) * CH)
                m8_sb = mtp.tile([P, HT, CH], F8, tag="mt")
                for ht in range(HT):
                    ps = ps_u.tile([P, CH], F, tag="u")
                    hsl = slice(ht * P, (ht + 1) * P)
                    passes = [(wfc8_t, h2T8), (dwfc8_t, h2T8), (wfc8_t, r8)]
                    for pa, (wt, act) in enumerate(passes):
                        for pr in range(DT // 2):
                            nc.tensor.matmul(
                                ps,
                                wt[:, 2 * pr:2 * pr + 2, hsl],
                                act[:, 2 * pr:2 * pr + 2, csl],
                                start=(pa == 0 and pr == 0),
                                stop=(pa == len(passes) - 1 and pr == DT // 2 - 1),
                                perf_mode=DR)
                    nc.scalar.activation(out=m8_sb[:, ht], in_=ps, func=AF.Gelu,
                                         bias=cols["bfc"][:, ht:ht + 1], scale=1.0 / SW)
                m8_sbs[sc] = m8_sb

            def proj(sc):
                m8_sb = m8_sbs[sc]
                for su in range(TPC):
                    st = sc * TPC + su
                    o2_t = phD.tile([P, D], F, tag="o2")
                    for dc in range(2):
                        ps = ps_p.tile([P, D2C], F, tag="o2p")
                        for pa, wt in enumerate((wpr8_t, dwpr8_t)):
                            for tr_ in range(HT // 2):
                                nc.tensor.matmul(
                                    ps,
                                    m8_sb[:, 2 * tr_:2 * tr_ + 2, su * P:(su + 1) * P],
                                    wt[:, 2 * tr_:2 * tr_ + 2, dc * D2C:(dc + 1) * D2C],
                                    start=(pa == 0 and tr_ == 0),
                                    stop=(pa == 1 and tr_ == HT // 2 - 1),
                                    perf_mode=DR)
                        dsl = slice(dc * D2C, (dc + 1) * D2C)
                        nc.vector.scalar_tensor_tensor(
                            out=o2_t[:, dsl], in0=ps, scalar=1.0 / SWP,
                            in1=x2bf[:, st, dsl], op0=OP.mult, op1=OP.add)
                    if not zero_bias:
                        nc.vector.tensor_tensor(out=o2_t, in0=o2_t, in1=bp_bc,
                                                op=OP.add)
                    nc.sync.dma_start(out_d.ap()[st * P:(st + 1) * P, :], o2_t)

            fc(0)
            for sc in range(1, NCH):
                fc(sc)
                proj(sc - 1)
            proj(NCH - 1)
        wmlp_ctx.close()

    return nc


def _get_nc(zero_bias=True, ln_trivial=True):
    key = ("nc", zero_bias, ln_trivial)
    if key not in _CACHE:
        nc = _build(zero_bias, ln_trivial)
        nc.compile()
        _CACHE[key] = nc
    return _CACHE[key]


TRACE = False


def kernel(**inputs):
    from concourse.bass_utils import run_bass_kernel_spmd

    zb, lt = build_flags(inputs)
    nc = _get_nc(zb, lt)
    x = np.asarray(inputs["x"], dtype=np.float32)
    base = host_inputs(inputs)
    in_maps = [dict(base, x=np.ascontiguousarray(x[b])) for b in range(N_CORES)]
    res = run_bass_kernel_spmd(nc, in_maps, core_ids=list(range(N_CORES)), trace=TRACE)
    _CACHE["last_res"] = res
    return np.stack([res.results[b]["out"] for b in range(N_CORES)], axis=0)


# revision 40
# speedup vs baseline: 1.1478x; 1.0728x over previous
"""Trainium2 Bass kernel for a dense transformer block (B=8, S=2048, D=768, H=3072).

Sharding: pure data-parallel over batch -- one batch element per NeuronCore.

All GEMMs run as fp8e4m3 DoubleRow matmuls (0.5 PE cycles per output row while
contracting 256 -- 4x the fp32r rate). Weights are quantized host-side with
power-of-2 scales (64x for D-sided weights, 128x for Wproj) so their sigma sits
mid-range in e4m3. Accuracy is recovered where it matters:
  - MLP fc: 3 passes  (W8@h8 + dW8@h8 + W8@r8), dW8/r8 = quantized residuals
  - MLP proj: 2 passes (P8@m8 + dP8@m8)
  - attention (qkv/scores/av/wo): single pass; softmax's diffuse weights make
    it insensitive to fp8 noise. exp is computed shifted (exp(s-2)) to fit
    e4m3's max-240 range.
Residual stream, LN statistics and all PSUM accumulation stay fp32/bf16.

Structure tricks:
  - LN outputs are transposed feature-major via XBAR dma_start_transpose
    (bf16, ~0.7us/tile on the SP/ACT HWDGE queues) instead of PE transposes
    + per-tile PSUM evacuation.
  - rsqrt for both LNs is exp(-0.5*ln(v+eps)) so the whole attention phase
    stays in ACT's natural_log_exp table -- no per-chunk table reloads.
  - qT8 is produced chunk-by-chunk inside the attention loop.

Engine balance (GPSIMD has no PSUM port, so PSUM consumers split ACT/DVE):
  ACT : exp, gelu, ln/exp-rsqrt, qT8 + v8 psum writes, half the transposes
  DVE : LN stats, kT8 psum writes, x2/out fused scale+add (stt), y8, rz
  Pool: LN normalize (SBUF), hT8/h2T8 fp8 copies + r8 residual, Wproj DMA
  SP  : x loads, Wq/Wk/Wo/Wfc DMA, half the transposes, out stores
"""

import numpy as np

P = 128
S, D, H = 2048, 768, 3072
DT = D // P            # 6 d-tiles
HT = H // P            # 24 h-tiles
ST = S // P            # 16 token tiles
CH = 512               # s1 chunk width
NCH = S // CH          # 4 chunks
TPC = CH // P          # 4 token tiles per chunk
D2C = 384              # half-D psum tile
EPS = 1e-5
N_CORES = 8

SW = 64.0              # host scale for Wq/Wk/Wv/Wo/Wfc
SWP = 128.0            # host scale for Wproj
EXP_SHIFT = -2.0       # exp(s + EXP_SHIFT): keeps e8 below e4m3 max (240)
ZINV = 1.0 / 32.0      # "ones" matmul value; rz = 32/Z keeps y8 ~ sigma 1

WEIGHT_NAMES = [
    "ln1_g", "ln1_b", "ln2_g", "ln2_b",
    "Wq", "bq", "Wk", "bk", "Wv", "bv", "Wo", "bo",
    "Wfc", "bfc", "Wproj", "bproj",
]

_CACHE = {}


def host_inputs(inputs):
    """Quantize weights host-side; returns the per-core shared tensor map."""
    import ml_dtypes
    F8 = ml_dtypes.float8_e4m3
    f32 = lambda a: np.ascontiguousarray(np.asarray(a, dtype=np.float32))
    q8 = lambda a: np.ascontiguousarray(np.asarray(a, dtype=np.float32).astype(F8))
    d = {nm: f32(inputs[nm]) for nm in WEIGHT_NAMES}
    m = {}
    for nm in ("Wq", "Wk", "Wv", "Wo"):
        m[nm + "8"] = q8(SW * d[nm])
    wfc8 = q8(SW * d["Wfc"])
    m["Wfc8"] = wfc8
    m["dWfc8"] = q8(SW * d["Wfc"] - wfc8.astype(np.float32))
    wpr8 = q8(SWP * d["Wproj"])
    m["Wpr8"] = wpr8
    m["dWpr8"] = q8(SWP * d["Wproj"] - wpr8.astype(np.float32))
    m["bq2"] = f32(2.0 * d["bq"])
    m["bk2"] = f32(2.0 * d["bk"])
    for nm in ("bv", "bo", "bfc", "bproj", "ln1_g", "ln1_b", "ln2_g", "ln2_b"):
        m[nm] = d[nm]
    return m


def build_flags(inputs):
    zb = all(float(np.abs(np.asarray(inputs[nm])).max()) == 0.0
             for nm in ("bv", "bo", "bproj"))
    lt = (float(np.abs(np.asarray(inputs["ln1_g"]) - 1.0).max()) == 0.0
          and float(np.abs(np.asarray(inputs["ln2_g"]) - 1.0).max()) == 0.0
          and float(np.abs(np.asarray(inputs["ln1_b"])).max()) == 0.0
          and float(np.abs(np.asarray(inputs["ln2_b"])).max()) == 0.0)
    return zb, lt


DRAM_SPECS = (
    [("Wq8", [D, D]), ("Wk8", [D, D]), ("Wv8", [D, D]), ("Wo8", [D, D]),
     ("Wfc8", [D, H]), ("dWfc8", [D, H]), ("Wpr8", [H, D]), ("dWpr8", [H, D])],
    [("bq2", [D]), ("bk2", [D]), ("bv", [D]), ("bo", [D]), ("bfc", [H]),
     ("bproj", [D]), ("ln1_g", [D]), ("ln1_b", [D]), ("ln2_g", [D]),
     ("ln2_b", [D])],
)


def _build(zero_bias=True, ln_trivial=True):
    import concourse.bass as bass
    import concourse.tile as tile
    from concourse import bacc, mybir
    from contextlib import ExitStack

    F = mybir.dt.float32
    BF = mybir.dt.bfloat16
    F8 = mybir.dt.float8e4
    AF = mybir.ActivationFunctionType
    OP = mybir.AluOpType
    DR = mybir.MatmulPerfMode.DoubleRow

    nc = bacc.Bacc(None, target_bir_lowering=False)

    x_d = nc.dram_tensor("x", [S, D], F, kind="ExternalInput")
    w_d = {}
    for nm, shp in DRAM_SPECS[0]:
        w_d[nm] = nc.dram_tensor(nm, shp, F8, kind="ExternalInput")
    for nm, shp in DRAM_SPECS[1]:
        w_d[nm] = nc.dram_tensor(nm, shp, F, kind="ExternalInput")
    out_d = nc.dram_tensor("out", [S, D], F, kind="ExternalOutput")

    def bcast_ap(dram_t, n_part=P):
        ap = dram_t.ap()
        return bass.AP(tensor=ap.tensor, offset=ap.offset, ap=[[0, n_part]] + list(ap.ap))

    inv_sqrt_d = 1.0 / float(np.sqrt(np.float32(D)))

    def wload(dst, name, eng=None, dtile=None):
        """Load weight [K, N] -> [P, K/P, N]; optionally one K-tile slice."""
        src = w_d[name].ap()
        if dtile is not None:
            src = src[dtile * P:(dtile + 1) * P, :]
            dst = dst[:, dtile:dtile + 1, :]
        (eng or nc.gpsimd).dma_start(dst, src.rearrange("(t p) n -> p t n", p=P))

    I32 = mybir.dt.int32
    RSQRT_MAGIC = 0x5F3759DF

    def rsqrt4(pool, rs_out, var_col, n):
        # Quake rsqrt on DVE (magic bit-hack + one Newton step, |err| < 0.2%
        # -- far below the fp8 noise floor), batched over n tiles' variances
        # so the dependency chain amortizes. Keeps sqrt off ACT so the whole
        # attention phase runs in one activation table. eps (1e-5 on var~1.3)
        # is dropped: a 4e-6 relative effect.
        y0 = pool.tile([P, n], F, tag="y0")
        nc.vector.tensor_scalar(out=y0[:].bitcast(I32), in0=var_col.bitcast(I32),
                                scalar1=1, scalar2=None,
                                op0=OP.logical_shift_right)
        nc.vector.tensor_scalar(out=y0[:].bitcast(I32), in0=y0[:].bitcast(I32),
                                scalar1=-1, scalar2=RSQRT_MAGIC,
                                op0=OP.mult, op1=OP.add)
        s = pool.tile([P, n], F, tag="nsq")
        nc.vector.tensor_tensor(out=s, in0=y0, in1=y0, op=OP.mult)
        nc.vector.tensor_tensor(out=s, in0=s, in1=var_col, op=OP.mult)
        nc.vector.tensor_scalar(out=s, in0=s, scalar1=-0.5, scalar2=1.5,
                                op0=OP.mult, op1=OP.add)
        nc.vector.tensor_tensor(out=rs_out, in0=y0, in1=s, op=OP.mult)

    with tile.TileContext(nc) as tc, ExitStack() as ctx:
        singles = ctx.enter_context(tc.tile_pool(name="singles", bufs=1))

        # persistent constants
        ones8 = singles.tile([P, 2, P], F8)
        nc.vector.memset(ones8, ZINV)
        eps_t = singles.tile([P, 1], F)
        nc.vector.memset(eps_t, EPS)
        eshift_t = singles.tile([P, 1], F)
        nc.vector.memset(eshift_t, EXP_SHIFT)
        cols = {}
        for nm, n in [("bq2", DT), ("bk2", DT), ("bfc", HT)]:
            t = singles.tile([P, n], F, tag=f"col_{nm}", name=f"col_{nm}")
            nc.sync.dma_start(t, w_d[nm].ap().rearrange("(t p) -> p t", p=P))
            cols[nm] = t
        ln_bc = {}
        if not ln_trivial:
            for nm in ("ln1_g", "ln1_b", "ln2_g", "ln2_b"):
                t = singles.tile([P, D], F, tag=f"bc_{nm}", name=f"bc_{nm}")
                nc.gpsimd.dma_start(out=t, in_=bcast_ap(w_d[nm]))
                ln_bc[nm] = t

        # persistent activation tensors (allocated lazily -- see pool notes)
        live = ctx.enter_context(tc.tile_pool(name="live", bufs=1))

        # wfc opens before kqv so kqv can close first (LIFO pool order)
        wmlp_ctx = ExitStack()
        wmlp = wmlp_ctx.enter_context(tc.tile_pool(name="wfc", bufs=1))

        attn_ctx = ExitStack()
        kqv = attn_ctx.enter_context(tc.tile_pool(name="kqv", bufs=1))
        kT8 = kqv.tile([P, DT, S], F8)
        qT8 = kqv.tile([P, DT, S], F8)
        v8 = kqv.tile([P, ST, D], F8)

        def ln_stats(pool, src_ap, mv_out):
            stats = pool.tile([P, 3, 6], F, tag="st")
            for i in range(3):
                nc.vector.bn_stats(out=stats[:, i, :],
                                   in_=src_ap[:, i * 256:(i + 1) * 256])
            nc.vector.bn_aggr(out=mv_out, in_=stats)

        def ln_norm(pool, src_ap, mu_col, rs_col, g_bc, b_bc):
            h_t = pool.tile([P, D], BF, tag="ht")
            nc.gpsimd.tensor_scalar(out=h_t, in0=src_ap, scalar1=mu_col,
                                    scalar2=rs_col, op0=OP.subtract, op1=OP.mult)
            if g_bc is not None:
                nc.gpsimd.tensor_tensor(out=h_t, in0=h_t, in1=g_bc, op=OP.mult)
                nc.gpsimd.tensor_tensor(out=h_t, in0=h_t, in1=b_bc, op=OP.add)
            return h_t

        # ---------------- Phase A: LN1 -> hT8; k,v ----------------
        with (
            tc.tile_pool(name="phA", bufs=3) as phA,
            tc.tile_pool(name="xtp", bufs=7) as xtp,
            tc.tile_pool(name="htr", bufs=4) as htrp,
            tc.tile_pool(name="hT", bufs=1) as hTp,
            tc.tile_pool(name="wqkv", bufs=1) as wqkv,
            tc.tile_pool(name="psAb", bufs=3, space="PSUM") as psAb,
        ):
            # hT8 and Wq survive into the attention block (per-chunk qT8)
            hT8 = kqv.tile([P, DT, S], F8)
            bv_bc = None
            if not zero_bias:
                bv_bc = hTp.tile([P, D], F)
                nc.gpsimd.dma_start(out=bv_bc, in_=bcast_ap(w_d["bv"]))
            wv8_t = wqkv.tile([P, DT, D], F8, tag="wv")
            wload(wv8_t, "Wv8")                       # Pool: needed first
            wq8_t = kqv.tile([P, DT, D], F8, tag="wq")
            wk8_t = wqkv.tile([P, DT, D], F8, tag="wk")
            wo8_t = kqv.tile([P, DT, D], F8)
            mv_all = hTp.tile([P, ST, 2], F)
            rs_all = hTp.tile([P, ST], F)

            # 3-stage software pipeline: [load+stats] -> (batched rsqrt)
            # -> [normalize+transpose+fp8] -> [v matmuls]
            x_ts = [None] * ST
            for st in range(ST + 6):
                if st == 2:
                    wload(wk8_t, "Wk8")               # Pool queue
                    wload(wq8_t, "Wq8")
                    wload(wo8_t, "Wo8")
                if st < ST:
                    x_t = xtp.tile([P, D], F, tag="xt")
                    nc.sync.dma_start(x_t, x_d.ap()[st * P:(st + 1) * P, :])
                    ln_stats(phA, x_t, mv_all[:, st, :])
                    x_ts[st] = x_t
                    if st % 4 == 3:
                        g = st - 3
                        rsqrt4(phA, rs_all[:, g:g + 4],
                               mv_all[:, g:g + 4, 1:2], 4)
                if 4 <= st < ST + 4:
                    sp = st - 4
                    h_t = ln_norm(phA, x_ts[sp], mv_all[:, sp, 0:1],
                                  rs_all[:, sp:sp + 1],
                                  ln_bc.get("ln1_g"), ln_bc.get("ln1_b"))
                    htr = htrp.tile([P, DT, P], BF, tag="htr")
                    eng = nc.sync if st % 2 == 0 else nc.scalar
                    eng.dma_start_transpose(htr, h_t)
                    nc.gpsimd.tensor_copy(
                        out=hT8[:, :, sp * P:(sp + 1) * P], in_=htr)
                if st >= 6:
                    sv = st - 6
                    for dc in range(2):
                        ps = psAb.tile([P, D2C], F, tag="mmv")
                        for pr in range(DT // 2):
                            nc.tensor.matmul(
                                ps,
                                hT8[:, 2 * pr:2 * pr + 2, sv * P:(sv + 1) * P],
                                wv8_t[:, 2 * pr:2 * pr + 2, dc * D2C:(dc + 1) * D2C],
                                start=(pr == 0), stop=(pr == DT // 2 - 1),
                                perf_mode=DR)
                        dsl = slice(dc * D2C, (dc + 1) * D2C)
                        if zero_bias:
                            nc.scalar.activation(out=v8[:, sv, dsl], in_=ps,
                                                 func=AF.Identity, scale=1.0 / SW)
                        else:
                            nc.vector.scalar_tensor_tensor(
                                out=v8[:, sv, dsl], in0=ps, scalar=1.0 / SW,
                                in1=bv_bc[:, dsl], op0=OP.mult, op1=OP.add)
            # k: phase B needs kT8 chunks in order; DVE evacuates
            for sc in range(NCH):
                for dtp in range(DT):
                    ps = psAb.tile([P, CH], F, tag="mm")
                    for pr in range(DT // 2):
                        nc.tensor.matmul(
                            ps,
                            wk8_t[:, 2 * pr:2 * pr + 2, dtp * P:(dtp + 1) * P],
                            hT8[:, 2 * pr:2 * pr + 2, sc * CH:(sc + 1) * CH],
                            start=(pr == 0), stop=(pr == DT // 2 - 1),
                            perf_mode=DR)
                    nc.vector.tensor_scalar(out=kT8[:, dtp, sc * CH:(sc + 1) * CH],
                                            in0=ps, scalar1=2.0 / SW,
                                            scalar2=cols["bk2"][:, dtp:dtp + 1],
                                            op0=OP.mult, op1=OP.add)

        # fc weights + persistent attention-output tensors
        wfc8_t = wmlp.tile([P, DT, H], F8)
        dwfc8_t = wmlp.tile([P, DT, H], F8)
        x2bf = live.tile([P, ST, D], BF)         # post-attn residual (bf16)
        h2T8 = live.tile([P, DT, S], F8)         # LN2 out, feature-major fp8
        r8 = live.tile([P, DT, S], F8)           # fp8 residual of h2T
        bo_bc = None
        if not zero_bias:
            bo_bc = live.tile([P, D], F)
            nc.gpsimd.dma_start(out=bo_bc, in_=bcast_ap(w_d["bo"]))

        # ---------------- Phase B/C: attention + fused LN2 ----------------
        NPR = ST // 2  # 8 s2 pairs
        with (
            tc.tile_pool(name="phC", bufs=2) as phC,
            tc.tile_pool(name="h2tr", bufs=3) as h2trp,
            tc.tile_pool(name="e8p", bufs=1) as e8p,
            tc.tile_pool(name="yt", bufs=2) as ytp,
            tc.tile_pool(name="ps_a", bufs=2, space="PSUM") as ps_a,
            tc.tile_pool(name="ps_y", bufs=6, space="PSUM") as ps_y,
        ):
            y8_sbs = [None] * NCH
            for sc in range(NCH + 1):
                if sc < NCH:
                    # qT8 for this chunk (ACT evacuates; identity rides the
                    # exp table so no act-table reload)
                    for dtp in range(DT):
                        ps = ps_a.tile([P, CH], F, tag="sc", name="ps_q")
                        for pr in range(DT // 2):
                            nc.tensor.matmul(
                                ps,
                                wq8_t[:, 2 * pr:2 * pr + 2, dtp * P:(dtp + 1) * P],
                                hT8[:, 2 * pr:2 * pr + 2, sc * CH:(sc + 1) * CH],
                                start=(pr == 0), stop=(pr == DT // 2 - 1),
                                perf_mode=DR)
                        nc.scalar.activation(out=qT8[:, dtp, sc * CH:(sc + 1) * CH],
                                             in_=ps, func=AF.Identity,
                                             bias=cols["bq2"][:, dtp:dtp + 1],
                                             scale=2.0 / SW)
                    e8c = e8p.tile([P, ST, CH], F8, tag="e8")
                    ps_ys = [ps_y.tile([P, CH], F, tag="y", name=f"ps_y{i}")
                             for i in range(DT)]
                    for pi in range(NPR + 1):
                        if pi < NPR:
                            for j in range(2):
                                st2 = 2 * pi + j
                                ps = ps_a.tile([P, CH], F, tag="sc")
                                for pr in range(DT // 2):
                                    nc.tensor.matmul(
                                        ps,
                                        kT8[:, 2 * pr:2 * pr + 2, st2 * P:(st2 + 1) * P],
                                        qT8[:, 2 * pr:2 * pr + 2, sc * CH:(sc + 1) * CH],
                                        start=(pr == 0), stop=(pr == DT // 2 - 1),
                                        perf_mode=DR)
                                nc.scalar.activation(out=e8c[:, st2, :], in_=ps,
                                                     func=AF.Exp, bias=eshift_t,
                                                     scale=inv_sqrt_d / 4.0)
                        if pi >= 1:
                            pp = pi - 1
                            for dtp in range(DT):
                                nc.tensor.matmul(
                                    ps_ys[dtp],
                                    v8[:, 2 * pp:2 * pp + 2, dtp * P:(dtp + 1) * P],
                                    e8c[:, 2 * pp:2 * pp + 2, :],
                                    start=(pp == 0), stop=(pp == NPR - 1),
                                    perf_mode=DR)
                    ps_zt = ps_a.tile([P, CH], F, tag="sc", name="ps_zt")
                    for pp in range(NPR):
                        nc.tensor.matmul(ps_zt, ones8, e8c[:, 2 * pp:2 * pp + 2, :],
                                         start=(pp == 0), stop=(pp == NPR - 1),
                                         perf_mode=DR)
                    rz = phC.tile([P, CH], F, tag="rz")
                    nc.vector.reciprocal(out=rz, in_=ps_zt)
                    y8_sb = ytp.tile([P, DT, CH], F8, tag="yt")
                    for dtp in range(DT):
                        nc.vector.tensor_tensor(out=y8_sb[:, dtp], in0=ps_ys[dtp],
                                                in1=rz, op=OP.mult)
                    y8_sbs[sc] = y8_sb

                if sc >= 1:
                    cc = sc - 1
                    y8_sb = y8_sbs[cc]
                    mv4 = phC.tile([P, TPC, 2], F, tag="mv4")
                    rs4 = phC.tile([P, TPC], F, tag="rs4")
                    for su in range(TPC):
                        st = cc * TPC + su
                        x_t = phC.tile([P, D], F, tag="xt3")
                        nc.sync.dma_start(x_t, x_d.ap()[st * P:(st + 1) * P, :])
                        # one Wfc d-tile slice per su rides the SP queue
                        if cc < 3:
                            k = cc * TPC + su
                            if k < DT:
                                wload(wfc8_t, "Wfc8", eng=nc.sync, dtile=k)
                            elif k < 2 * DT:
                                wload(dwfc8_t, "dWfc8", eng=nc.sync, dtile=k - DT)
                        for dc in range(2):
                            ps = ps_y.tile([P, D2C], F, tag="y", name="ps_o")
                            for pr in range(DT // 2):
                                nc.tensor.matmul(
                                    ps,
                                    y8_sb[:, 2 * pr:2 * pr + 2, su * P:(su + 1) * P],
                                    wo8_t[:, 2 * pr:2 * pr + 2, dc * D2C:(dc + 1) * D2C],
                                    start=(pr == 0), stop=(pr == DT // 2 - 1),
                                    perf_mode=DR)
                            dsl = slice(dc * D2C, (dc + 1) * D2C)
                            nc.vector.scalar_tensor_tensor(
                                out=x2bf[:, st, dsl], in0=ps,
                                scalar=1.0 / (32.0 * SW), in1=x_t[:, dsl],
                                op0=OP.mult, op1=OP.add)
                        if not zero_bias:
                            nc.vector.tensor_tensor(out=x2bf[:, st, :],
                                                    in0=x2bf[:, st, :],
                                                    in1=bo_bc, op=OP.add)
                        ln_stats(phC, x2bf[:, st, :], mv4[:, su, :])
                    rsqrt4(phC, rs4, mv4[:, :, 1:2], TPC)
                    for su in range(TPC):
                        st = cc * TPC + su
                        h2_t = ln_norm(phC, x2bf[:, st, :], mv4[:, su, 0:1],
                                       rs4[:, su:su + 1],
                                       ln_bc.get("ln2_g"), ln_bc.get("ln2_b"))
                        # XBAR transpose needs a dense destination (strided
                        # dest is silently wrong on HW) -- land in h2tr, then
                        # Pool writes the fp8 copy + residual.
                        h2tr = h2trp.tile([P, DT, P], BF, tag="h2tr")
                        eng = nc.sync if su % 2 == 0 else nc.scalar
                        eng.dma_start_transpose(h2tr, h2_t)
                        tsl = slice(st * P, (st + 1) * P)
                        nc.gpsimd.tensor_copy(out=h2T8[:, :, tsl], in_=h2tr)
                        nc.gpsimd.tensor_tensor(out=r8[:, :, tsl], in0=h2tr,
                                                in1=h2T8[:, :, tsl],
                                                op=OP.subtract)

        attn_ctx.close()

        # ---------------- Phase D: MLP ----------------
        wpr_pool = wmlp_ctx.enter_context(tc.tile_pool(name="wpr", bufs=1))
        wpr8_t = wpr_pool.tile([P, HT, D], F8)
        dwpr8_t = wpr_pool.tile([P, HT, D], F8)
        bp_bc = None
        if not zero_bias:
            bp_bc = wpr_pool.tile([P, D], F)
            nc.gpsimd.dma_start(out=bp_bc, in_=bcast_ap(w_d["bproj"]))
        with (
            tc.tile_pool(name="phD", bufs=2) as phD,
            tc.tile_pool(name="mt", bufs=2) as mtp,
            tc.tile_pool(name="ps_u", bufs=4, space="PSUM") as ps_u,
            tc.tile_pool(name="ps_p", bufs=4, space="PSUM") as ps_p,
        ):
            m8_sbs = [None] * NCH

            def fc(sc):
                csl = slice(sc * CH, (sc + 1) * CH)
                m8_sb = mtp.tile([P, HT, CH], F8, tag="mt")
                for ht in range(HT):
                    ps = ps_u.tile([P, CH], F, tag="u")
                    hsl = slice(ht * P, (ht + 1) * P)
                    passes = [(wfc8_t, h2T8), (dwfc8_t, h2T8), (wfc8_t, r8)]
                    for pa, (wt, act) in enumerate(passes):
                        for pr in range(DT // 2):
                            nc.tensor.matmul(
                                ps,
                                wt[:, 2 * pr:2 * pr + 2, hsl],
                                act[:, 2 * pr:2 * pr + 2, csl],
                                start=(pa == 0 and pr == 0),
                                stop=(pa == len(passes) - 1 and pr == DT // 2 - 1),
                                perf_mode=DR)
                    nc.scalar.activation(out=m8_sb[:, ht], in_=ps, func=AF.Gelu,
                                         bias=cols["bfc"][:, ht:ht + 1], scale=1.0 / SW)
                m8_sbs[sc] = m8_sb

            def proj(sc):
                m8_sb = m8_sbs[sc]
                for su in range(TPC):
                    st = sc * TPC + su
                    o2_t = phD.tile([P, D], F, tag="o2")
                    for dc in range(2):
                        ps = ps_p.tile([P, D2C], F, tag="o2p")
                        for pa, wt in enumerate((wpr8_t, dwpr8_t)):
                            for tr_ in range(HT // 2):
                                nc.tensor.matmul(
                                    ps,
                                    m8_sb[:, 2 * tr_:2 * tr_ + 2, su * P:(su + 1) * P],
                                    wt[:, 2 * tr_:2 * tr_ + 2, dc * D2C:(dc + 1) * D2C],
                                    start=(pa == 0 and tr_ == 0),
                                    stop=(pa == 1 and tr_ == HT // 2 - 1),
                                    perf_mode=DR)
                        dsl = slice(dc * D2C, (dc + 1) * D2C)
                        nc.vector.scalar_tensor_tensor(
                            out=o2_t[:, dsl], in0=ps, scalar=1.0 / SWP,
                            in1=x2bf[:, st, dsl], op0=OP.mult, op1=OP.add)
                    if not zero_bias:
                        nc.vector.tensor_tensor(out=o2_t, in0=o2_t, in1=bp_bc,
                                                op=OP.add)
                    nc.sync.dma_start(out_d.ap()[st * P:(st + 1) * P, :], o2_t)

            fc(0)
            # proj weights stream on SP while fc(0)/fc(1) run on PE
            wload(wpr8_t, "Wpr8", eng=nc.sync)
            wload(dwpr8_t, "dWpr8", eng=nc.sync)
            for sc in range(1, NCH):
                fc(sc)
                proj(sc - 1)
            proj(NCH - 1)
        wmlp_ctx.close()

    return nc


def _get_nc(zero_bias=True, ln_trivial=True):
    key = ("nc", zero_bias, ln_trivial)
    if key not in _CACHE:
        nc = _build(zero_bias, ln_trivial)
        nc.compile()
        _CACHE[key] = nc
    return _CACHE[key]


TRACE = False


def kernel(**inputs):
    from concourse.bass_utils import run_bass_kernel_spmd

    zb, lt = build_flags(inputs)
    nc = _get_nc(zb, lt)
    x = np.asarray(inputs["x"], dtype=np.float32)
    base = host_inputs(inputs)
    in_maps = [dict(base, x=np.ascontiguousarray(x[b])) for b in range(N_CORES)]
    res = run_bass_kernel_spmd(nc, in_maps, core_ids=list(range(N_CORES)), trace=TRACE)
    _CACHE["last_res"] = res
    return np.stack([res.results[b]["out"] for b in range(N_CORES)], axis=0)


# revision 44
# speedup vs baseline: 1.1637x; 1.0138x over previous
"""Trainium2 Bass kernel for a dense transformer block (B=8, S=2048, D=768, H=3072).

Sharding: pure data-parallel over batch -- one batch element per NeuronCore.

All GEMMs run as fp8e4m3 DoubleRow matmuls (0.5 PE cycles per output row while
contracting 256 -- 4x the fp32r rate). Weights are quantized host-side with
power-of-2 scales (64x for D-sided weights, 128x for Wproj) so their sigma sits
mid-range in e4m3. Accuracy is recovered where it matters:
  - MLP fc: 3 passes  (W8@h8 + dW8@h8 + W8@r8), dW8/r8 = quantized residuals
  - MLP proj: 2 passes (P8@m8 + dP8@m8)
  - attention (qkv/scores/av/wo): single pass; softmax's diffuse weights make
    it insensitive to fp8 noise. exp is computed shifted (exp(s-2)) to fit
    e4m3's max-240 range.
Residual stream, LN statistics and all PSUM accumulation stay fp32/bf16.

Structure tricks:
  - LN outputs are transposed feature-major via XBAR dma_start_transpose
    (bf16, ~0.7us/tile on the SP/ACT HWDGE queues) instead of PE transposes
    + per-tile PSUM evacuation.
  - rsqrt for both LNs is exp(-0.5*ln(v+eps)) so the whole attention phase
    stays in ACT's natural_log_exp table -- no per-chunk table reloads.
  - qT8 is produced chunk-by-chunk inside the attention loop.

Engine balance (GPSIMD has no PSUM port, so PSUM consumers split ACT/DVE):
  ACT : exp, gelu, ln/exp-rsqrt, qT8 + v8 psum writes, half the transposes
  DVE : LN stats, kT8 psum writes, x2/out fused scale+add (stt), y8, rz
  Pool: LN normalize (SBUF), hT8/h2T8 fp8 copies + r8 residual, Wproj DMA
  SP  : x loads, Wq/Wk/Wo/Wfc DMA, half the transposes, out stores
"""

import numpy as np

P = 128
S, D, H = 2048, 768, 3072
DT = D // P            # 6 d-tiles
HT = H // P            # 24 h-tiles
ST = S // P            # 16 token tiles
CH = 512               # s1 chunk width
NCH = S // CH          # 4 chunks
TPC = CH // P          # 4 token tiles per chunk
D2C = 384              # half-D psum tile
EPS = 1e-5
N_CORES = 8

SW = 64.0              # host scale for Wq/Wk/Wv/Wo/Wfc
SWP = 128.0            # host scale for Wproj
EXP_SHIFT = -2.0       # exp(s + EXP_SHIFT): keeps e8 below e4m3 max (240)
ZINV = 1.0 / 32.0      # "ones" matmul value; rz = 32/Z keeps y8 ~ sigma 1

WEIGHT_NAMES = [
    "ln1_g", "ln1_b", "ln2_g", "ln2_b",
    "Wq", "bq", "Wk", "bk", "Wv", "bv", "Wo", "bo",
    "Wfc", "bfc", "Wproj", "bproj",
]

_CACHE = {}


def host_inputs(inputs):
    """Quantize weights host-side; returns the per-core shared tensor map."""
    import ml_dtypes
    F8 = ml_dtypes.float8_e4m3
    f32 = lambda a: np.ascontiguousarray(np.asarray(a, dtype=np.float32))
    q8 = lambda a: np.ascontiguousarray(np.asarray(a, dtype=np.float32).astype(F8))
    d = {nm: f32(inputs[nm]) for nm in WEIGHT_NAMES}
    m = {}
    for nm in ("Wq", "Wk", "Wv", "Wo"):
        m[nm + "8"] = q8(SW * d[nm])
    wfc8 = q8(SW * d["Wfc"])
    m["Wfc8"] = wfc8
    m["dWfc8"] = q8(SW * d["Wfc"] - wfc8.astype(np.float32))
    wpr8 = q8(SWP * d["Wproj"])
    m["Wpr8"] = wpr8
    m["dWpr8"] = q8(SWP * d["Wproj"] - wpr8.astype(np.float32))
    m["bq2"] = f32(2.0 * d["bq"])
    m["bk2"] = f32(2.0 * d["bk"])
    for nm in ("bv", "bo", "bfc", "bproj", "ln1_g", "ln1_b", "ln2_g", "ln2_b"):
        m[nm] = d[nm]
    return m


def build_flags(inputs):
    zb = all(float(np.abs(np.asarray(inputs[nm])).max()) == 0.0
             for nm in ("bv", "bo", "bproj"))
    lt = (float(np.abs(np.asarray(inputs["ln1_g"]) - 1.0).max()) == 0.0
          and float(np.abs(np.asarray(inputs["ln2_g"]) - 1.0).max()) == 0.0
          and float(np.abs(np.asarray(inputs["ln1_b"])).max()) == 0.0
          and float(np.abs(np.asarray(inputs["ln2_b"])).max()) == 0.0)
    return zb, lt


DRAM_SPECS = (
    [("Wq8", [D, D]), ("Wk8", [D, D]), ("Wv8", [D, D]), ("Wo8", [D, D]),
     ("Wfc8", [D, H]), ("dWfc8", [D, H]), ("Wpr8", [H, D]), ("dWpr8", [H, D])],
    [("bq2", [D]), ("bk2", [D]), ("bv", [D]), ("bo", [D]), ("bfc", [H]),
     ("bproj", [D]), ("ln1_g", [D]), ("ln1_b", [D]), ("ln2_g", [D]),
     ("ln2_b", [D])],
)


def _build(zero_bias=True, ln_trivial=True):
    import concourse.bass as bass
    import concourse.tile as tile
    from concourse import bacc, mybir
    from contextlib import ExitStack

    F = mybir.dt.float32
    BF = mybir.dt.bfloat16
    F8 = mybir.dt.float8e4
    AF = mybir.ActivationFunctionType
    OP = mybir.AluOpType
    DR = mybir.MatmulPerfMode.DoubleRow

    nc = bacc.Bacc(None, target_bir_lowering=False)

    x_d = nc.dram_tensor("x", [S, D], F, kind="ExternalInput")
    w_d = {}
    for nm, shp in DRAM_SPECS[0]:
        w_d[nm] = nc.dram_tensor(nm, shp, F8, kind="ExternalInput")
    for nm, shp in DRAM_SPECS[1]:
        w_d[nm] = nc.dram_tensor(nm, shp, F, kind="ExternalInput")
    out_d = nc.dram_tensor("out", [S, D], F, kind="ExternalOutput")

    def bcast_ap(dram_t, n_part=P):
        ap = dram_t.ap()
        return bass.AP(tensor=ap.tensor, offset=ap.offset, ap=[[0, n_part]] + list(ap.ap))

    inv_sqrt_d = 1.0 / float(np.sqrt(np.float32(D)))

    def wload(dst, name, eng=None, dtile=None):
        """Load weight [K, N] -> [P, K/P, N]; optionally one K-tile slice."""
        src = w_d[name].ap()
        if dtile is not None:
            src = src[dtile * P:(dtile + 1) * P, :]
            dst = dst[:, dtile:dtile + 1, :]
        (eng or nc.gpsimd).dma_start(dst, src.rearrange("(t p) n -> p t n", p=P))

    I32 = mybir.dt.int32
    RSQRT_MAGIC = 0x5F3759DF

    def rsqrt4(pool, rs_out, var_col, n):
        # Quake rsqrt on DVE (magic bit-hack + one Newton step, |err| < 0.2%
        # -- far below the fp8 noise floor), batched over n tiles' variances
        # so the dependency chain amortizes. Keeps sqrt off ACT so the whole
        # attention phase runs in one activation table. eps (1e-5 on var~1.3)
        # is dropped: a 4e-6 relative effect.
        y0 = pool.tile([P, n], F, tag="y0")
        nc.vector.tensor_scalar(out=y0[:].bitcast(I32), in0=var_col.bitcast(I32),
                                scalar1=1, scalar2=None,
                                op0=OP.logical_shift_right)
        nc.vector.tensor_scalar(out=y0[:].bitcast(I32), in0=y0[:].bitcast(I32),
                                scalar1=-1, scalar2=RSQRT_MAGIC,
                                op0=OP.mult, op1=OP.add)
        s = pool.tile([P, n], F, tag="nsq")
        nc.vector.tensor_tensor(out=s, in0=y0, in1=y0, op=OP.mult)
        nc.vector.tensor_tensor(out=s, in0=s, in1=var_col, op=OP.mult)
        nc.vector.tensor_scalar(out=s, in0=s, scalar1=-0.5, scalar2=1.5,
                                op0=OP.mult, op1=OP.add)
        nc.vector.tensor_tensor(out=rs_out, in0=y0, in1=s, op=OP.mult)

    with tile.TileContext(nc) as tc, ExitStack() as ctx:
        singles = ctx.enter_context(tc.tile_pool(name="singles", bufs=1))

        # persistent constants
        ones8 = singles.tile([P, 2, P], F8)
        nc.vector.memset(ones8, ZINV)
        eps_t = singles.tile([P, 1], F)
        nc.vector.memset(eps_t, EPS)
        eshift_t = singles.tile([P, 1], F)
        nc.vector.memset(eshift_t, EXP_SHIFT)
        cols = {}
        for nm, n in [("bq2", DT), ("bk2", DT), ("bfc", HT)]:
            t = singles.tile([P, n], F, tag=f"col_{nm}", name=f"col_{nm}")
            nc.sync.dma_start(t, w_d[nm].ap().rearrange("(t p) -> p t", p=P))
            cols[nm] = t
        ln_bc = {}
        if not ln_trivial:
            for nm in ("ln1_g", "ln1_b", "ln2_g", "ln2_b"):
                t = singles.tile([P, D], F, tag=f"bc_{nm}", name=f"bc_{nm}")
                nc.gpsimd.dma_start(out=t, in_=bcast_ap(w_d[nm]))
                ln_bc[nm] = t

        # persistent activation tensors (allocated lazily -- see pool notes)
        live = ctx.enter_context(tc.tile_pool(name="live", bufs=1))

        # wfc opens before kqv so kqv can close first (LIFO pool order)
        wmlp_ctx = ExitStack()
        wmlp = wmlp_ctx.enter_context(tc.tile_pool(name="wfc", bufs=1))

        attn_ctx = ExitStack()
        kqv = attn_ctx.enter_context(tc.tile_pool(name="kqv", bufs=1))
        kT8 = kqv.tile([P, DT, S], F8)
        qT8 = kqv.tile([P, DT, S], F8)
        v8 = kqv.tile([P, ST, D], F8)

        def ln_stats(pool, src_ap, mv_out):
            stats = pool.tile([P, 3, 6], F, tag="st")
            for i in range(3):
                nc.vector.bn_stats(out=stats[:, i, :],
                                   in_=src_ap[:, i * 256:(i + 1) * 256])
            nc.vector.bn_aggr(out=mv_out, in_=stats)

        def ln_norm(pool, src_ap, mu_col, rs_col, g_bc, b_bc):
            h_t = pool.tile([P, D], BF, tag="ht")
            nc.gpsimd.tensor_scalar(out=h_t, in0=src_ap, scalar1=mu_col,
                                    scalar2=rs_col, op0=OP.subtract, op1=OP.mult)
            if g_bc is not None:
                nc.gpsimd.tensor_tensor(out=h_t, in0=h_t, in1=g_bc, op=OP.mult)
                nc.gpsimd.tensor_tensor(out=h_t, in0=h_t, in1=b_bc, op=OP.add)
            return h_t

        # ---------------- Phase A: LN1 -> hT8; k,v ----------------
        with (
            tc.tile_pool(name="phA", bufs=3) as phA,
            tc.tile_pool(name="xtp", bufs=7) as xtp,
            tc.tile_pool(name="htr", bufs=4) as htrp,
            tc.tile_pool(name="hT", bufs=1) as hTp,
            tc.tile_pool(name="wqkv", bufs=1) as wqkv,
            tc.tile_pool(name="psAb", bufs=3, space="PSUM") as psAb,
        ):
            # hT8 and Wq survive into the attention block (per-chunk qT8)
            hT8 = kqv.tile([P, DT, S], F8)
            bv_bc = None
            if not zero_bias:
                bv_bc = hTp.tile([P, D], F)
                nc.gpsimd.dma_start(out=bv_bc, in_=bcast_ap(w_d["bv"]))
            wv8_t = wqkv.tile([P, DT, D], F8, tag="wv")
            wload(wv8_t, "Wv8")                       # Pool: needed first
            wq8_t = kqv.tile([P, DT, D], F8, tag="wq")
            wk8_t = wqkv.tile([P, DT, D], F8, tag="wk")
            wo8_t = kqv.tile([P, DT, D], F8)
            mv_all = hTp.tile([P, ST, 2], F)
            rs_all = hTp.tile([P, ST], F)

            # 3-stage software pipeline: [load+stats] -> (batched rsqrt)
            # -> [normalize+transpose+fp8] -> [v matmuls]
            x_ts = [None] * ST
            for st in range(ST + 6):
                if st == 2:
                    wload(wk8_t, "Wk8")               # Pool queue
                    wload(wq8_t, "Wq8")
                    wload(wo8_t, "Wo8")
                if st < ST:
                    x_t = xtp.tile([P, D], F, tag="xt")
                    xeng = nc.scalar if st % 2 == 0 else nc.sync
                    xeng.dma_start(x_t, x_d.ap()[st * P:(st + 1) * P, :])
                    ln_stats(phA, x_t, mv_all[:, st, :])
                    x_ts[st] = x_t
                    if st % 4 == 3:
                        g = st - 3
                        rsqrt4(phA, rs_all[:, g:g + 4],
                               mv_all[:, g:g + 4, 1:2], 4)
                if 4 <= st < ST + 4:
                    sp = st - 4
                    h_t = ln_norm(phA, x_ts[sp], mv_all[:, sp, 0:1],
                                  rs_all[:, sp:sp + 1],
                                  ln_bc.get("ln1_g"), ln_bc.get("ln1_b"))
                    htr = htrp.tile([P, DT, P], BF, tag="htr")
                    eng = nc.sync if st % 2 == 0 else nc.scalar
                    eng.dma_start_transpose(htr, h_t)
                    nc.gpsimd.tensor_copy(
                        out=hT8[:, :, sp * P:(sp + 1) * P], in_=htr)
                if st >= 6:
                    sv = st - 6
                    for dc in range(2):
                        ps = psAb.tile([P, D2C], F, tag="mmv")
                        for pr in range(DT // 2):
                            nc.tensor.matmul(
                                ps,
                                hT8[:, 2 * pr:2 * pr + 2, sv * P:(sv + 1) * P],
                                wv8_t[:, 2 * pr:2 * pr + 2, dc * D2C:(dc + 1) * D2C],
                                start=(pr == 0), stop=(pr == DT // 2 - 1),
                                perf_mode=DR)
                        dsl = slice(dc * D2C, (dc + 1) * D2C)
                        if zero_bias:
                            nc.scalar.activation(out=v8[:, sv, dsl], in_=ps,
                                                 func=AF.Identity, scale=1.0 / SW)
                        else:
                            nc.vector.scalar_tensor_tensor(
                                out=v8[:, sv, dsl], in0=ps, scalar=1.0 / SW,
                                in1=bv_bc[:, dsl], op0=OP.mult, op1=OP.add)
            # k: phase B needs kT8 chunks in order; DVE evacuates
            for sc in range(NCH):
                for dtp in range(DT):
                    ps = psAb.tile([P, CH], F, tag="mm")
                    for pr in range(DT // 2):
                        nc.tensor.matmul(
                            ps,
                            wk8_t[:, 2 * pr:2 * pr + 2, dtp * P:(dtp + 1) * P],
                            hT8[:, 2 * pr:2 * pr + 2, sc * CH:(sc + 1) * CH],
                            start=(pr == 0), stop=(pr == DT // 2 - 1),
                            perf_mode=DR)
                    nc.vector.tensor_scalar(out=kT8[:, dtp, sc * CH:(sc + 1) * CH],
                                            in0=ps, scalar1=2.0 / SW,
                                            scalar2=cols["bk2"][:, dtp:dtp + 1],
                                            op0=OP.mult, op1=OP.add)

        # fc weights + persistent attention-output tensors
        wfc8_t = wmlp.tile([P, DT, H], F8)
        dwfc8_t = wmlp.tile([P, DT, H], F8)
        x2bf = live.tile([P, ST, D], BF)         # post-attn residual (bf16)
        h2T8 = live.tile([P, DT, S], F8)         # LN2 out, feature-major fp8
        r8 = live.tile([P, DT, S], F8)           # fp8 residual of h2T
        bo_bc = None
        if not zero_bias:
            bo_bc = live.tile([P, D], F)
            nc.gpsimd.dma_start(out=bo_bc, in_=bcast_ap(w_d["bo"]))

        # ---------------- Phase B/C: attention + fused LN2 ----------------
        NPR = ST // 2  # 8 s2 pairs
        with (
            tc.tile_pool(name="phC", bufs=2) as phC,
            tc.tile_pool(name="h2tr", bufs=3) as h2trp,
            tc.tile_pool(name="e8p", bufs=1) as e8p,
            tc.tile_pool(name="yt", bufs=2) as ytp,
            tc.tile_pool(name="ps_a", bufs=2, space="PSUM") as ps_a,
            tc.tile_pool(name="ps_y", bufs=3, space="PSUM") as ps_y,
        ):
            # B/C holds only 5 PSUM banks (2 score + 3 yT-accum) so phase D's
            # fc accumulators can open in fresh banks and start the moment
            # their SBUF inputs exist -- not when the attention pools drain.
            y8_sbs = [None] * NCH
            HDT = DT // 2
            for sc in range(NCH + 1):
                if sc < NCH:
                    # qT8 for this chunk (ACT evacuates; identity rides the
                    # exp table so no act-table reload)
                    for dtp in range(DT):
                        ps = ps_a.tile([P, CH], F, tag="sc", name="ps_q")
                        for pr in range(DT // 2):
                            nc.tensor.matmul(
                                ps,
                                wq8_t[:, 2 * pr:2 * pr + 2, dtp * P:(dtp + 1) * P],
                                hT8[:, 2 * pr:2 * pr + 2, sc * CH:(sc + 1) * CH],
                                start=(pr == 0), stop=(pr == DT // 2 - 1),
                                perf_mode=DR)
                        nc.scalar.activation(out=qT8[:, dtp, sc * CH:(sc + 1) * CH],
                                             in_=ps, func=AF.Identity,
                                             bias=cols["bq2"][:, dtp:dtp + 1],
                                             scale=2.0 / SW)
                    e8c = e8p.tile([P, ST, CH], F8, tag="e8")
                    y8_sb = ytp.tile([P, DT, CH], F8, tag="yt")
                    # pass 1: scores+exp interleaved with yT accumulation for
                    # d-tiles 0-2 (3 psum banks); pass 2 re-streams e8c for 3-5
                    ps_ys = [ps_y.tile([P, CH], F, tag="y", name=f"ps_y{i}")
                             for i in range(HDT)]
                    for pi in range(NPR + 1):
                        if pi < NPR:
                            for j in range(2):
                                st2 = 2 * pi + j
                                ps = ps_a.tile([P, CH], F, tag="sc")
                                for pr in range(HDT):
                                    nc.tensor.matmul(
                                        ps,
                                        kT8[:, 2 * pr:2 * pr + 2, st2 * P:(st2 + 1) * P],
                                        qT8[:, 2 * pr:2 * pr + 2, sc * CH:(sc + 1) * CH],
                                        start=(pr == 0), stop=(pr == HDT - 1),
                                        perf_mode=DR)
                                nc.scalar.activation(out=e8c[:, st2, :], in_=ps,
                                                     func=AF.Exp, bias=eshift_t,
                                                     scale=inv_sqrt_d / 4.0)
                        if pi >= 1:
                            pp = pi - 1
                            for dtp in range(HDT):
                                nc.tensor.matmul(
                                    ps_ys[dtp],
                                    v8[:, 2 * pp:2 * pp + 2, dtp * P:(dtp + 1) * P],
                                    e8c[:, 2 * pp:2 * pp + 2, :],
                                    start=(pp == 0), stop=(pp == NPR - 1),
                                    perf_mode=DR)
                    ps_zt = ps_a.tile([P, CH], F, tag="sc", name="ps_zt")
                    for pp in range(NPR):
                        nc.tensor.matmul(ps_zt, ones8, e8c[:, 2 * pp:2 * pp + 2, :],
                                         start=(pp == 0), stop=(pp == NPR - 1),
                                         perf_mode=DR)
                    rz = phC.tile([P, CH], F, tag="rz")
                    nc.vector.reciprocal(out=rz, in_=ps_zt)
                    for dtp in range(HDT):
                        nc.vector.tensor_tensor(out=y8_sb[:, dtp], in0=ps_ys[dtp],
                                                in1=rz, op=OP.mult)
                    ps_ys2 = [ps_y.tile([P, CH], F, tag="y", name=f"ps_y{i}")
                              for i in range(HDT)]
                    for pp in range(NPR):
                        for dtp in range(HDT):
                            nc.tensor.matmul(
                                ps_ys2[dtp],
                                v8[:, 2 * pp:2 * pp + 2, (HDT + dtp) * P:(HDT + dtp + 1) * P],
                                e8c[:, 2 * pp:2 * pp + 2, :],
                                start=(pp == 0), stop=(pp == NPR - 1),
                                perf_mode=DR)
                    for dtp in range(HDT):
                        nc.vector.tensor_tensor(out=y8_sb[:, HDT + dtp],
                                                in0=ps_ys2[dtp],
                                                in1=rz, op=OP.mult)
                    y8_sbs[sc] = y8_sb

                if sc >= 1:
                    cc = sc - 1
                    y8_sb = y8_sbs[cc]
                    mv4 = phC.tile([P, TPC, 2], F, tag="mv4")
                    rs4 = phC.tile([P, TPC], F, tag="rs4")
                    for su in range(TPC):
                        st = cc * TPC + su
                        x_t = phC.tile([P, D], F, tag="xt3")
                        nc.sync.dma_start(x_t, x_d.ap()[st * P:(st + 1) * P, :])
                        # one Wfc d-tile slice per su rides the SP queue
                        if cc < 3:
                            k = cc * TPC + su
                            if k < DT:
                                wload(wfc8_t, "Wfc8", eng=nc.sync, dtile=k)
                            elif k < 2 * DT:
                                wload(dwfc8_t, "dWfc8", eng=nc.sync, dtile=k - DT)
                        for dc in range(2):
                            ps = ps_y.tile([P, D2C], F, tag="y", name="ps_o")
                            for pr in range(DT // 2):
                                nc.tensor.matmul(
                                    ps,
                                    y8_sb[:, 2 * pr:2 * pr + 2, su * P:(su + 1) * P],
                                    wo8_t[:, 2 * pr:2 * pr + 2, dc * D2C:(dc + 1) * D2C],
                                    start=(pr == 0), stop=(pr == DT // 2 - 1),
                                    perf_mode=DR)
                            dsl = slice(dc * D2C, (dc + 1) * D2C)
                            nc.vector.scalar_tensor_tensor(
                                out=x2bf[:, st, dsl], in0=ps,
                                scalar=1.0 / (32.0 * SW), in1=x_t[:, dsl],
                                op0=OP.mult, op1=OP.add)
                        if not zero_bias:
                            nc.vector.tensor_tensor(out=x2bf[:, st, :],
                                                    in0=x2bf[:, st, :],
                                                    in1=bo_bc, op=OP.add)
                        ln_stats(phC, x2bf[:, st, :], mv4[:, su, :])
                    rsqrt4(phC, rs4, mv4[:, :, 1:2], TPC)
                    for su in range(TPC):
                        st = cc * TPC + su
                        h2_t = ln_norm(phC, x2bf[:, st, :], mv4[:, su, 0:1],
                                       rs4[:, su:su + 1],
                                       ln_bc.get("ln2_g"), ln_bc.get("ln2_b"))
                        # XBAR transpose needs a dense destination (strided
                        # dest is silently wrong on HW) -- land in h2tr, then
                        # Pool writes the fp8 copy + residual.
                        h2tr = h2trp.tile([P, DT, P], BF, tag="h2tr")
                        eng = nc.sync if su % 2 == 0 else nc.scalar
                        eng.dma_start_transpose(h2tr, h2_t)
                        tsl = slice(st * P, (st + 1) * P)
                        nc.gpsimd.tensor_copy(out=h2T8[:, :, tsl], in_=h2tr)
                        nc.gpsimd.tensor_tensor(out=r8[:, :, tsl], in0=h2tr,
                                                in1=h2T8[:, :, tsl],
                                                op=OP.subtract)

        attn_ctx.close()

        # ---------------- Phase D: MLP ----------------
        wpr_pool = wmlp_ctx.enter_context(tc.tile_pool(name="wpr", bufs=1))
        wpr8_t = wpr_pool.tile([P, HT, D], F8)
        dwpr8_t = wpr_pool.tile([P, HT, D], F8)
        bp_bc = None
        if not zero_bias:
            bp_bc = wpr_pool.tile([P, D], F)
            nc.gpsimd.dma_start(out=bp_bc, in_=bcast_ap(w_d["bproj"]))
        with (
            tc.tile_pool(name="phD", bufs=2) as phD,
            tc.tile_pool(name="mt", bufs=2) as mtp,
            tc.tile_pool(name="ps_u", bufs=3, space="PSUM") as ps_u,
            tc.tile_pool(name="ps_p", bufs=3, space="PSUM") as ps_p,
        ):
            m8_sbs = [None] * NCH

            def fc(sc):
                csl = slice(sc * CH, (sc + 1) * CH)
                m8_sb = mtp.tile([P, HT, CH], F8, tag="mt")
                for ht in range(HT):
                    ps = ps_u.tile([P, CH], F, tag="u")
                    hsl = slice(ht * P, (ht + 1) * P)
                    passes = [(wfc8_t, h2T8), (dwfc8_t, h2T8), (wfc8_t, r8)]
                    for pa, (wt, act) in enumerate(passes):
                        for pr in range(DT // 2):
                            nc.tensor.matmul(
                                ps,
                                wt[:, 2 * pr:2 * pr + 2, hsl],
                                act[:, 2 * pr:2 * pr + 2, csl],
                                start=(pa == 0 and pr == 0),
                                stop=(pa == len(passes) - 1 and pr == DT // 2 - 1),
                                perf_mode=DR)
                    nc.scalar.activation(out=m8_sb[:, ht], in_=ps, func=AF.Gelu,
                                         bias=cols["bfc"][:, ht:ht + 1], scale=1.0 / SW)
                m8_sbs[sc] = m8_sb

            def proj(sc):
                m8_sb = m8_sbs[sc]
                for su in range(TPC):
                    st = sc * TPC + su
                    o2_t = phD.tile([P, D], F, tag="o2")
                    for dc in range(2):
                        ps = ps_p.tile([P, D2C], F, tag="o2p")
                        for pa, wt in enumerate((wpr8_t, dwpr8_t)):
                            for tr_ in range(HT // 2):
                                nc.tensor.matmul(
                                    ps,
                                    m8_sb[:, 2 * tr_:2 * tr_ + 2, su * P:(su + 1) * P],
                                    wt[:, 2 * tr_:2 * tr_ + 2, dc * D2C:(dc + 1) * D2C],
                                    start=(pa == 0 and tr_ == 0),
                                    stop=(pa == 1 and tr_ == HT // 2 - 1),
                                    perf_mode=DR)
                        dsl = slice(dc * D2C, (dc + 1) * D2C)
                        nc.vector.scalar_tensor_tensor(
                            out=o2_t[:, dsl], in0=ps, scalar=1.0 / SWP,
                            in1=x2bf[:, st, dsl], op0=OP.mult, op1=OP.add)
                    if not zero_bias:
                        nc.vector.tensor_tensor(out=o2_t, in0=o2_t, in1=bp_bc,
                                                op=OP.add)
                    nc.sync.dma_start(out_d.ap()[st * P:(st + 1) * P, :], o2_t)

            fc(0)
            # proj weights stream on SP while fc(0)/fc(1) run on PE
            wload(wpr8_t, "Wpr8", eng=nc.sync)
            wload(dwpr8_t, "dWpr8", eng=nc.sync)
            for sc in range(1, NCH):
                fc(sc)
                proj(sc - 1)
            proj(NCH - 1)
        wmlp_ctx.close()

    return nc


def _get_nc(zero_bias=True, ln_trivial=True):
    key = ("nc", zero_bias, ln_trivial)
    if key not in _CACHE:
        nc = _build(zero_bias, ln_trivial)
        nc.compile()
        _CACHE[key] = nc
    return _CACHE[key]


TRACE = False


def kernel(**inputs):
    from concourse.bass_utils import run_bass_kernel_spmd

    zb, lt = build_flags(inputs)
    nc = _get_nc(zb, lt)
    x = np.asarray(inputs["x"], dtype=np.float32)
    base = host_inputs(inputs)
    in_maps = [dict(base, x=np.ascontiguousarray(x[b])) for b in range(N_CORES)]
    res = run_bass_kernel_spmd(nc, in_maps, core_ids=list(range(N_CORES)), trace=TRACE)
    _CACHE["last_res"] = res
    return np.stack([res.results[b]["out"] for b in range(N_CORES)], axis=0)


# revision 50
# speedup vs baseline: 1.2439x; 1.0690x over previous
"""Trainium2 Bass kernel for a dense transformer block (B=8, S=2048, D=768, H=3072).

Sharding: pure data-parallel over batch -- one batch element per NeuronCore.

All GEMMs run as fp8e4m3 DoubleRow matmuls (0.5 PE cycles per output row while
contracting 256 -- 4x the fp32r rate). Weights are quantized host-side with
power-of-2 scales (64x for D-sided weights, 128x for Wproj) so their sigma sits
mid-range in e4m3. Accuracy is recovered where it matters:
  - MLP fc: 3 passes  (W8@h8 + dW8@h8 + W8@r8), dW8/r8 = quantized residuals
  - MLP proj: 2 passes (P8@m8 + dP8@m8)
  - attention (qkv/scores/av/wo): single pass; softmax's diffuse weights make
    it insensitive to fp8 noise. exp is computed shifted (exp(s-2)) to fit
    e4m3's max-240 range.
Residual stream, LN statistics and all PSUM accumulation stay fp32/bf16.

Structure tricks:
  - LN outputs are transposed feature-major via XBAR dma_start_transpose
    (bf16, ~0.7us/tile on the SP/ACT HWDGE queues) instead of PE transposes
    + per-tile PSUM evacuation.
  - rsqrt for both LNs is exp(-0.5*ln(v+eps)) so the whole attention phase
    stays in ACT's natural_log_exp table -- no per-chunk table reloads.
  - qT8 is produced chunk-by-chunk inside the attention loop.

Engine balance (GPSIMD has no PSUM port, so PSUM consumers split ACT/DVE):
  ACT : exp, gelu, ln/exp-rsqrt, qT8 + v8 psum writes, half the transposes
  DVE : LN stats, kT8 psum writes, x2/out fused scale+add (stt), y8, rz
  Pool: LN normalize (SBUF), hT8/h2T8 fp8 copies + r8 residual, Wproj DMA
  SP  : x loads, Wq/Wk/Wo/Wfc DMA, half the transposes, out stores
"""

import numpy as np

P = 128
S, D, H = 2048, 768, 3072
DT = D // P            # 6 d-tiles
HT = H // P            # 24 h-tiles
ST = S // P            # 16 token tiles
CH = 512               # s1 chunk width
NCH = S // CH          # 4 chunks
TPC = CH // P          # 4 token tiles per chunk
D2C = 384              # half-D psum tile
EPS = 1e-5
N_CORES = 8

SW = 64.0              # host scale for Wq/Wk/Wv/Wo/Wfc
SWP = 128.0            # host scale for Wproj
EXP_SHIFT = -2.0       # exp(s + EXP_SHIFT): keeps e8 below e4m3 max (240)
ZINV = 1.0 / 32.0      # "ones" matmul value; rz = 32/Z keeps y8 ~ sigma 1

WEIGHT_NAMES = [
    "ln1_g", "ln1_b", "ln2_g", "ln2_b",
    "Wq", "bq", "Wk", "bk", "Wv", "bv", "Wo", "bo",
    "Wfc", "bfc", "Wproj", "bproj",
]

_CACHE = {}


def host_inputs(inputs):
    """Quantize weights host-side; returns the per-core shared tensor map."""
    import ml_dtypes
    F8 = ml_dtypes.float8_e4m3
    f32 = lambda a: np.ascontiguousarray(np.asarray(a, dtype=np.float32))
    q8 = lambda a: np.ascontiguousarray(np.asarray(a, dtype=np.float32).astype(F8))
    d = {nm: f32(inputs[nm]) for nm in WEIGHT_NAMES}
    m = {}
    for nm in ("Wq", "Wk", "Wv", "Wo"):
        m[nm + "8"] = q8(SW * d[nm])
    wfc8 = q8(SW * d["Wfc"])
    m["Wfc8"] = wfc8
    m["dWfc8"] = q8(SW * d["Wfc"] - wfc8.astype(np.float32))
    wpr8 = q8(SWP * d["Wproj"])
    m["Wpr8"] = wpr8
    m["dWpr8"] = q8(SWP * d["Wproj"] - wpr8.astype(np.float32))
    m["bq2"] = f32(2.0 * d["bq"])
    m["bk2"] = f32(2.0 * d["bk"])
    for nm in ("bv", "bo", "bfc", "bproj", "ln1_g", "ln1_b", "ln2_g", "ln2_b"):
        m[nm] = d[nm]
    return m


def build_flags(inputs):
    zb = all(float(np.abs(np.asarray(inputs[nm])).max()) == 0.0
             for nm in ("bv", "bo", "bproj"))
    lt = (float(np.abs(np.asarray(inputs["ln1_g"]) - 1.0).max()) == 0.0
          and float(np.abs(np.asarray(inputs["ln2_g"]) - 1.0).max()) == 0.0
          and float(np.abs(np.asarray(inputs["ln1_b"])).max()) == 0.0
          and float(np.abs(np.asarray(inputs["ln2_b"])).max()) == 0.0)
    return zb, lt


DRAM_SPECS = (
    [("Wq8", [D, D]), ("Wk8", [D, D]), ("Wv8", [D, D]), ("Wo8", [D, D]),
     ("Wfc8", [D, H]), ("dWfc8", [D, H]), ("Wpr8", [H, D]), ("dWpr8", [H, D])],
    [("bq2", [D]), ("bk2", [D]), ("bv", [D]), ("bo", [D]), ("bfc", [H]),
     ("bproj", [D]), ("ln1_g", [D]), ("ln1_b", [D]), ("ln2_g", [D]),
     ("ln2_b", [D])],
)


def _build(zero_bias=True, ln_trivial=True):
    import concourse.bass as bass
    import concourse.tile as tile
    from concourse import bacc, mybir
    from contextlib import ExitStack

    F = mybir.dt.float32
    BF = mybir.dt.bfloat16
    F8 = mybir.dt.float8e4
    AF = mybir.ActivationFunctionType
    OP = mybir.AluOpType
    DR = mybir.MatmulPerfMode.DoubleRow

    nc = bacc.Bacc(None, target_bir_lowering=False)

    x_d = nc.dram_tensor("x", [S, D], F, kind="ExternalInput")
    w_d = {}
    for nm, shp in DRAM_SPECS[0]:
        w_d[nm] = nc.dram_tensor(nm, shp, F8, kind="ExternalInput")
    for nm, shp in DRAM_SPECS[1]:
        w_d[nm] = nc.dram_tensor(nm, shp, F, kind="ExternalInput")
    out_d = nc.dram_tensor("out", [S, D], F, kind="ExternalOutput")

    def bcast_ap(dram_t, n_part=P):
        ap = dram_t.ap()
        return bass.AP(tensor=ap.tensor, offset=ap.offset, ap=[[0, n_part]] + list(ap.ap))

    inv_sqrt_d = 1.0 / float(np.sqrt(np.float32(D)))

    def wload(dst, name, eng=None, dtile=None):
        """Load weight [K, N] -> [P, K/P, N]; optionally one K-tile slice."""
        src = w_d[name].ap()
        if dtile is not None:
            src = src[dtile * P:(dtile + 1) * P, :]
            dst = dst[:, dtile:dtile + 1, :]
        (eng or nc.gpsimd).dma_start(dst, src.rearrange("(t p) n -> p t n", p=P))

    I32 = mybir.dt.int32
    RSQRT_MAGIC = 0x5F3759DF

    def rsqrt4(pool, rs_out, var_col, n):
        # Quake rsqrt on DVE (magic bit-hack + one Newton step, |err| < 0.2%
        # -- far below the fp8 noise floor), batched over n tiles' variances
        # so the dependency chain amortizes. Keeps sqrt off ACT so the whole
        # attention phase runs in one activation table. eps (1e-5 on var~1.3)
        # is dropped: a 4e-6 relative effect.
        y0 = pool.tile([P, n], F, tag="y0")
        nc.vector.tensor_scalar(out=y0[:].bitcast(I32), in0=var_col.bitcast(I32),
                                scalar1=1, scalar2=None,
                                op0=OP.logical_shift_right)
        nc.vector.tensor_scalar(out=y0[:].bitcast(I32), in0=y0[:].bitcast(I32),
                                scalar1=-1, scalar2=RSQRT_MAGIC,
                                op0=OP.mult, op1=OP.add)
        s = pool.tile([P, n], F, tag="nsq")
        nc.vector.tensor_tensor(out=s, in0=y0, in1=y0, op=OP.mult)
        nc.vector.tensor_tensor(out=s, in0=s, in1=var_col, op=OP.mult)
        nc.vector.tensor_scalar(out=s, in0=s, scalar1=-0.5, scalar2=1.5,
                                op0=OP.mult, op1=OP.add)
        nc.vector.tensor_tensor(out=rs_out, in0=y0, in1=s, op=OP.mult)

    with tile.TileContext(nc) as tc, ExitStack() as ctx:
        singles = ctx.enter_context(tc.tile_pool(name="singles", bufs=1))

        # persistent constants
        ones8 = singles.tile([P, 2, P], F8)
        nc.vector.memset(ones8, ZINV)
        eps_t = singles.tile([P, 1], F)
        nc.vector.memset(eps_t, EPS)
        eshift_t = singles.tile([P, 1], F)
        nc.vector.memset(eshift_t, EXP_SHIFT)
        cols = {}
        for nm, n in [("bq2", DT), ("bk2", DT), ("bfc", HT)]:
            t = singles.tile([P, n], F, tag=f"col_{nm}", name=f"col_{nm}")
            nc.sync.dma_start(t, w_d[nm].ap().rearrange("(t p) -> p t", p=P))
            cols[nm] = t
        ln_bc = {}
        if not ln_trivial:
            for nm in ("ln1_g", "ln1_b", "ln2_g", "ln2_b"):
                t = singles.tile([P, D], F, tag=f"bc_{nm}", name=f"bc_{nm}")
                nc.gpsimd.dma_start(out=t, in_=bcast_ap(w_d[nm]))
                ln_bc[nm] = t

        # persistent activation tensors (allocated lazily -- see pool notes)
        live = ctx.enter_context(tc.tile_pool(name="live", bufs=1))
        # m8 pool opens before the attention pools so its bytes never overlap
        # them: fc(0) must not wait for the attention epilogue to drain.
        mtp = ctx.enter_context(tc.tile_pool(name="mt", bufs=2))

        # wfc opens before kqv so kqv can close first (LIFO pool order)
        wmlp_ctx = ExitStack()
        wmlp = wmlp_ctx.enter_context(tc.tile_pool(name="wfc", bufs=1))

        attn_ctx = ExitStack()
        kqv = attn_ctx.enter_context(tc.tile_pool(name="kqv", bufs=1))
        kT8 = kqv.tile([P, DT, S], F8)
        v8 = kqv.tile([P, ST, D], F8)

        def ln_stats(pool, src_ap, mv_out):
            stats = pool.tile([P, 3, 6], F, tag="st")
            for i in range(3):
                nc.vector.bn_stats(out=stats[:, i, :],
                                   in_=src_ap[:, i * 256:(i + 1) * 256])
            nc.vector.bn_aggr(out=mv_out, in_=stats)

        def ln_norm(pool, src_ap, mu_col, rs_col, g_bc, b_bc):
            h_t = pool.tile([P, D], BF, tag="ht")
            nc.gpsimd.tensor_scalar(out=h_t, in0=src_ap, scalar1=mu_col,
                                    scalar2=rs_col, op0=OP.subtract, op1=OP.mult)
            if g_bc is not None:
                nc.gpsimd.tensor_tensor(out=h_t, in0=h_t, in1=g_bc, op=OP.mult)
                nc.gpsimd.tensor_tensor(out=h_t, in0=h_t, in1=b_bc, op=OP.add)
            return h_t

        # ---------------- Phase A: LN1 -> hT8; k,v ----------------
        with (
            tc.tile_pool(name="phA", bufs=3) as phA,
            tc.tile_pool(name="xtp", bufs=7) as xtp,
            tc.tile_pool(name="htr", bufs=4) as htrp,
            tc.tile_pool(name="hT", bufs=1) as hTp,
            tc.tile_pool(name="wqkv", bufs=1) as wqkv,
            tc.tile_pool(name="psAb", bufs=3, space="PSUM") as psAb,
        ):
            # hT8 and Wq survive into the attention block (per-chunk qT8)
            hT8 = kqv.tile([P, DT, S], F8)
            bv_bc = None
            if not zero_bias:
                bv_bc = hTp.tile([P, D], F)
                nc.gpsimd.dma_start(out=bv_bc, in_=bcast_ap(w_d["bv"]))
            wv8_t = wqkv.tile([P, DT, D], F8, tag="wv")
            wload(wv8_t, "Wv8")                       # Pool: needed first
            wq8_t = kqv.tile([P, DT, D], F8, tag="wq")
            wk8_t = wqkv.tile([P, DT, D], F8, tag="wk")
            wo8_t = kqv.tile([P, DT, D], F8)
            mv_all = hTp.tile([P, ST, 2], F)
            rs_all = hTp.tile([P, ST], F)

            # 3-stage software pipeline: [load+stats] -> (batched rsqrt)
            # -> [normalize+transpose+fp8] -> [v matmuls]
            x_ts = [None] * ST
            for st in range(ST + 6):
                if st == 2:
                    wload(wk8_t, "Wk8")               # Pool queue
                    wload(wq8_t, "Wq8")
                    wload(wo8_t, "Wo8")
                if st < ST:
                    x_t = xtp.tile([P, D], F, tag="xt")
                    xeng = nc.scalar if st % 2 == 0 else nc.sync
                    xeng.dma_start(x_t, x_d.ap()[st * P:(st + 1) * P, :])
                    ln_stats(phA, x_t, mv_all[:, st, :])
                    x_ts[st] = x_t
                    if st % 4 == 3:
                        g = st - 3
                        rsqrt4(phA, rs_all[:, g:g + 4],
                               mv_all[:, g:g + 4, 1:2], 4)
                if 4 <= st < ST + 4:
                    sp = st - 4
                    h_t = ln_norm(phA, x_ts[sp], mv_all[:, sp, 0:1],
                                  rs_all[:, sp:sp + 1],
                                  ln_bc.get("ln1_g"), ln_bc.get("ln1_b"))
                    htr = htrp.tile([P, DT, P], BF, tag="htr")
                    eng = nc.sync if st % 2 == 0 else nc.scalar
                    eng.dma_start_transpose(htr, h_t)
                    nc.gpsimd.tensor_copy(
                        out=hT8[:, :, sp * P:(sp + 1) * P], in_=htr)
                if st >= 6:
                    sv = st - 6
                    for dc in range(2):
                        ps = psAb.tile([P, D2C], F, tag="mmv")
                        for pr in range(DT // 2):
                            nc.tensor.matmul(
                                ps,
                                hT8[:, 2 * pr:2 * pr + 2, sv * P:(sv + 1) * P],
                                wv8_t[:, 2 * pr:2 * pr + 2, dc * D2C:(dc + 1) * D2C],
                                start=(pr == 0), stop=(pr == DT // 2 - 1),
                                perf_mode=DR)
                        dsl = slice(dc * D2C, (dc + 1) * D2C)
                        if zero_bias:
                            nc.scalar.activation(out=v8[:, sv, dsl], in_=ps,
                                                 func=AF.Identity, scale=1.0 / SW)
                        else:
                            nc.vector.scalar_tensor_tensor(
                                out=v8[:, sv, dsl], in0=ps, scalar=1.0 / SW,
                                in1=bv_bc[:, dsl], op0=OP.mult, op1=OP.add)
            # k: phase B needs kT8 chunks in order; DVE evacuates
            for sc in range(NCH):
                for dtp in range(DT):
                    ps = psAb.tile([P, CH], F, tag="mm")
                    for pr in range(DT // 2):
                        nc.tensor.matmul(
                            ps,
                            wk8_t[:, 2 * pr:2 * pr + 2, dtp * P:(dtp + 1) * P],
                            hT8[:, 2 * pr:2 * pr + 2, sc * CH:(sc + 1) * CH],
                            start=(pr == 0), stop=(pr == DT // 2 - 1),
                            perf_mode=DR)
                    nc.vector.tensor_scalar(out=kT8[:, dtp, sc * CH:(sc + 1) * CH],
                                            in0=ps, scalar1=2.0 / SW,
                                            scalar2=cols["bk2"][:, dtp:dtp + 1],
                                            op0=OP.mult, op1=OP.add)

        # fc weights + persistent attention-output tensors
        wfc8_t = wmlp.tile([P, DT, H], F8)
        dwfc8_t = wmlp.tile([P, DT, H], F8)
        x2bf = live.tile([P, ST, D], BF)         # post-attn residual (bf16)
        h2T8 = live.tile([P, DT, S], F8)         # LN2 out, feature-major fp8
        r8 = live.tile([P, DT, S], F8)           # fp8 residual of h2T
        bo_bc = None
        if not zero_bias:
            bo_bc = live.tile([P, D], F)
            nc.gpsimd.dma_start(out=bo_bc, in_=bcast_ap(w_d["bo"]))

        # ---------------- Phase B/C: attention + fused LN2 ----------------
        NPR = ST // 2  # 8 s2 pairs
        with (
            tc.tile_pool(name="phC", bufs=2) as phC,
            tc.tile_pool(name="h2tr", bufs=3) as h2trp,
            tc.tile_pool(name="e8p", bufs=1) as e8p,
            tc.tile_pool(name="qc8", bufs=2) as qc8p,
            tc.tile_pool(name="yt", bufs=2) as ytp,
            tc.tile_pool(name="ps_a", bufs=2, space="PSUM") as ps_a,
            tc.tile_pool(name="ps_y", bufs=3, space="PSUM") as ps_y,
        ):
            # B/C holds only 5 PSUM banks (2 score + 3 yT-accum) so phase D's
            # fc accumulators can open in fresh banks and start the moment
            # their SBUF inputs exist -- not when the attention pools drain.
            y8_sbs = [None] * NCH
            HDT = DT // 2
            for sc in range(NCH + 1):
                if sc < NCH:
                    # qT8 for this chunk (ACT evacuates; identity rides the
                    # exp table so no act-table reload)
                    qT8 = qc8p.tile([P, DT, CH], F8, tag="qc8")
                    for dtp in range(DT):
                        ps = ps_a.tile([P, CH], F, tag="sc", name="ps_q")
                        for pr in range(DT // 2):
                            nc.tensor.matmul(
                                ps,
                                wq8_t[:, 2 * pr:2 * pr + 2, dtp * P:(dtp + 1) * P],
                                hT8[:, 2 * pr:2 * pr + 2, sc * CH:(sc + 1) * CH],
                                start=(pr == 0), stop=(pr == DT // 2 - 1),
                                perf_mode=DR)
                        nc.scalar.activation(out=qT8[:, dtp, :],
                                             in_=ps, func=AF.Identity,
                                             bias=cols["bq2"][:, dtp:dtp + 1],
                                             scale=2.0 / SW)
                    e8c = e8p.tile([P, ST, CH], F8, tag="e8")
                    y8_sb = ytp.tile([P, DT, CH], F8, tag="yt")
                    # pass 1: scores+exp interleaved with yT accumulation for
                    # d-tiles 0-2 (3 psum banks); pass 2 re-streams e8c for 3-5
                    ps_ys = [ps_y.tile([P, CH], F, tag="y", name=f"ps_y{i}")
                             for i in range(HDT)]
                    for pi in range(NPR + 1):
                        if pi < NPR:
                            for j in range(2):
                                st2 = 2 * pi + j
                                ps = ps_a.tile([P, CH], F, tag="sc")
                                for pr in range(HDT):
                                    nc.tensor.matmul(
                                        ps,
                                        kT8[:, 2 * pr:2 * pr + 2, st2 * P:(st2 + 1) * P],
                                        qT8[:, 2 * pr:2 * pr + 2, :],
                                        start=(pr == 0), stop=(pr == HDT - 1),
                                        perf_mode=DR)
                                nc.scalar.activation(out=e8c[:, st2, :], in_=ps,
                                                     func=AF.Exp, bias=eshift_t,
                                                     scale=inv_sqrt_d / 4.0)
                        if pi >= 1:
                            pp = pi - 1
                            for dtp in range(HDT):
                                nc.tensor.matmul(
                                    ps_ys[dtp],
                                    v8[:, 2 * pp:2 * pp + 2, dtp * P:(dtp + 1) * P],
                                    e8c[:, 2 * pp:2 * pp + 2, :],
                                    start=(pp == 0), stop=(pp == NPR - 1),
                                    perf_mode=DR)
                    ps_zt = ps_a.tile([P, CH], F, tag="sc", name="ps_zt")
                    for pp in range(NPR):
                        nc.tensor.matmul(ps_zt, ones8, e8c[:, 2 * pp:2 * pp + 2, :],
                                         start=(pp == 0), stop=(pp == NPR - 1),
                                         perf_mode=DR)
                    rz = phC.tile([P, CH], F, tag="rz")
                    nc.vector.reciprocal(out=rz, in_=ps_zt)
                    for dtp in range(HDT):
                        nc.vector.tensor_tensor(out=y8_sb[:, dtp], in0=ps_ys[dtp],
                                                in1=rz, op=OP.mult)
                    ps_ys2 = [ps_y.tile([P, CH], F, tag="y", name=f"ps_y{i}")
                              for i in range(HDT)]
                    for pp in range(NPR):
                        for dtp in range(HDT):
                            nc.tensor.matmul(
                                ps_ys2[dtp],
                                v8[:, 2 * pp:2 * pp + 2, (HDT + dtp) * P:(HDT + dtp + 1) * P],
                                e8c[:, 2 * pp:2 * pp + 2, :],
                                start=(pp == 0), stop=(pp == NPR - 1),
                                perf_mode=DR)
                    for dtp in range(HDT):
                        nc.vector.tensor_tensor(out=y8_sb[:, HDT + dtp],
                                                in0=ps_ys2[dtp],
                                                in1=rz, op=OP.mult)
                    y8_sbs[sc] = y8_sb

                if sc >= 1:
                    cc = sc - 1
                    y8_sb = y8_sbs[cc]
                    mv4 = phC.tile([P, TPC, 2], F, tag="mv4")
                    rs4 = phC.tile([P, TPC], F, tag="rs4")
                    for su in range(TPC):
                        st = cc * TPC + su
                        x_t = phC.tile([P, D], F, tag="xt3")
                        nc.sync.dma_start(x_t, x_d.ap()[st * P:(st + 1) * P, :])
                        # one Wfc d-tile slice per su rides the SP queue
                        if cc < 3:
                            k = cc * TPC + su
                            if k < DT:
                                wload(wfc8_t, "Wfc8", eng=nc.sync, dtile=k)
                            elif k < 2 * DT:
                                wload(dwfc8_t, "dWfc8", eng=nc.sync, dtile=k - DT)
                        for dc in range(2):
                            ps = ps_y.tile([P, D2C], F, tag="y", name="ps_o")
                            for pr in range(DT // 2):
                                nc.tensor.matmul(
                                    ps,
                                    y8_sb[:, 2 * pr:2 * pr + 2, su * P:(su + 1) * P],
                                    wo8_t[:, 2 * pr:2 * pr + 2, dc * D2C:(dc + 1) * D2C],
                                    start=(pr == 0), stop=(pr == DT // 2 - 1),
                                    perf_mode=DR)
                            dsl = slice(dc * D2C, (dc + 1) * D2C)
                            nc.vector.scalar_tensor_tensor(
                                out=x2bf[:, st, dsl], in0=ps,
                                scalar=1.0 / (32.0 * SW), in1=x_t[:, dsl],
                                op0=OP.mult, op1=OP.add)
                        if not zero_bias:
                            nc.vector.tensor_tensor(out=x2bf[:, st, :],
                                                    in0=x2bf[:, st, :],
                                                    in1=bo_bc, op=OP.add)
                        ln_stats(phC, x2bf[:, st, :], mv4[:, su, :])
                    rsqrt4(phC, rs4, mv4[:, :, 1:2], TPC)
                    for su in range(TPC):
                        st = cc * TPC + su
                        h2_t = ln_norm(phC, x2bf[:, st, :], mv4[:, su, 0:1],
                                       rs4[:, su:su + 1],
                                       ln_bc.get("ln2_g"), ln_bc.get("ln2_b"))
                        # XBAR transpose needs a dense destination (strided
                        # dest is silently wrong on HW) -- land in h2tr, then
                        # Pool writes the fp8 copy + residual.
                        h2tr = h2trp.tile([P, DT, P], BF, tag="h2tr")
                        eng = nc.sync if su % 2 == 0 else nc.scalar
                        eng.dma_start_transpose(h2tr, h2_t)
                        tsl = slice(st * P, (st + 1) * P)
                        nc.gpsimd.tensor_copy(out=h2T8[:, :, tsl], in_=h2tr)
                        nc.gpsimd.tensor_tensor(out=r8[:, :, tsl], in0=h2tr,
                                                in1=h2T8[:, :, tsl],
                                                op=OP.subtract)

        attn_ctx.close()

        # ---------------- Phase D: MLP ----------------
        wpr_pool = wmlp_ctx.enter_context(tc.tile_pool(name="wpr", bufs=1))
        wpr8_t = wpr_pool.tile([P, HT, D], F8)
        dwpr8_t = wpr_pool.tile([P, HT, D], F8)
        bp_bc = None
        if not zero_bias:
            bp_bc = wpr_pool.tile([P, D], F)
            nc.gpsimd.dma_start(out=bp_bc, in_=bcast_ap(w_d["bproj"]))
        with (
            tc.tile_pool(name="phD", bufs=2) as phD,
            tc.tile_pool(name="ps_u", bufs=3, space="PSUM") as ps_u,
            tc.tile_pool(name="ps_p", bufs=3, space="PSUM") as ps_p,
        ):
            m8_sbs = [None] * NCH

            def fc(sc):
                csl = slice(sc * CH, (sc + 1) * CH)
                m8_sb = mtp.tile([P, HT, CH], F8, tag="mt")
                for ht in range(HT):
                    ps = ps_u.tile([P, CH], F, tag="u")
                    hsl = slice(ht * P, (ht + 1) * P)
                    passes = [(wfc8_t, h2T8), (dwfc8_t, h2T8), (wfc8_t, r8)]
                    for pa, (wt, act) in enumerate(passes):
                        for pr in range(DT // 2):
                            nc.tensor.matmul(
                                ps,
                                wt[:, 2 * pr:2 * pr + 2, hsl],
                                act[:, 2 * pr:2 * pr + 2, csl],
                                start=(pa == 0 and pr == 0),
                                stop=(pa == len(passes) - 1 and pr == DT // 2 - 1),
                                perf_mode=DR)
                    nc.scalar.activation(out=m8_sb[:, ht], in_=ps, func=AF.Gelu,
                                         bias=cols["bfc"][:, ht:ht + 1], scale=1.0 / SW)
                m8_sbs[sc] = m8_sb

            def proj(sc):
                m8_sb = m8_sbs[sc]
                for su in range(TPC):
                    st = sc * TPC + su
                    o2_t = phD.tile([P, D], F, tag="o2")
                    for dc in range(2):
                        ps = ps_p.tile([P, D2C], F, tag="o2p")
                        for pa, wt in enumerate((wpr8_t, dwpr8_t)):
                            for tr_ in range(HT // 2):
                                nc.tensor.matmul(
                                    ps,
                                    m8_sb[:, 2 * tr_:2 * tr_ + 2, su * P:(su + 1) * P],
                                    wt[:, 2 * tr_:2 * tr_ + 2, dc * D2C:(dc + 1) * D2C],
                                    start=(pa == 0 and tr_ == 0),
                                    stop=(pa == 1 and tr_ == HT // 2 - 1),
                                    perf_mode=DR)
                        dsl = slice(dc * D2C, (dc + 1) * D2C)
                        nc.vector.scalar_tensor_tensor(
                            out=o2_t[:, dsl], in0=ps, scalar=1.0 / SWP,
                            in1=x2bf[:, st, dsl], op0=OP.mult, op1=OP.add)
                    if not zero_bias:
                        nc.vector.tensor_tensor(out=o2_t, in0=o2_t, in1=bp_bc,
                                                op=OP.add)
                    nc.sync.dma_start(out_d.ap()[st * P:(st + 1) * P, :], o2_t)

            fc(0)
            # proj weights stream on SP while fc(0)/fc(1) run on PE
            wload(wpr8_t, "Wpr8", eng=nc.sync)
            wload(dwpr8_t, "dWpr8", eng=nc.sync)
            for sc in range(1, NCH):
                fc(sc)
                proj(sc - 1)
            proj(NCH - 1)
        wmlp_ctx.close()

    return nc


def _get_nc(zero_bias=True, ln_trivial=True):
    key = ("nc", zero_bias, ln_trivial)
    if key not in _CACHE:
        nc = _build(zero_bias, ln_trivial)
        nc.compile()
        _CACHE[key] = nc
    return _CACHE[key]


TRACE = False


def kernel(**inputs):
    from concourse.bass_utils import run_bass_kernel_spmd

    zb, lt = build_flags(inputs)
    nc = _get_nc(zb, lt)
    x = np.asarray(inputs["x"], dtype=np.float32)
    base = host_inputs(inputs)
    in_maps = [dict(base, x=np.ascontiguousarray(x[b])) for b in range(N_CORES)]
    res = run_bass_kernel_spmd(nc, in_maps, core_ids=list(range(N_CORES)), trace=TRACE)
    _CACHE["last_res"] = res
    return np.stack([res.results[b]["out"] for b in range(N_CORES)], axis=0)
